# revision 1
# baseline (speedup 1.0000x reference)
"""Trainium2 Bass kernel for nn_Block_76519137345684 (Spikformer-style block:
spiking self-attention + spiking gated MLP with training-mode BatchNorm).

Strategy
- Data-parallel over batch B across 8 NeuronCores (16 batch each). BN batch
  statistics (per-channel sum / sum-of-squares) are AllReduced across cores.
- Activations live channel-on-partition: (C, rows) with rows
  r = ((t*16 + b)*64 + n); LIF timesteps are contiguous 1024-column slices.
- BN is applied as ONE ACT-engine pass per ptile after the stats AllReduce:
  yh = 0.5*a*z + 0.5*c  (a = g*rsqrt(var+eps), c = be - mu*a), f16 output.
  The LIF recurrence then has UNIFORM immediate thresholds:
      u_t = 0.5*u_{t-1}*[u_{t-1} < thr] + yh_t,   s_t = [u_t >= thr]
  implemented with tensor_scalar/tensor_tensor ops only (no per-partition
  scalar operands, no scalar_tensor_tensor) - those are the fast DVE paths.
- Attention uses associativity: y = q @ (k^T v) * scale; per-head block
  structure enforced with a 0.125-scaled block-diagonal mask.
- Depthwise 3x3 conv runs on the TENSOR engine: spikes are written into a
  zero-padded plane layout (10 rows x 12 cols per frame); 9 taps become 9
  PSUM-accumulated matmuls with diagonal per-channel weight matrices against
  shifted views of the plane. Valid positions are repacked on ACT.
- Matmul dtypes: f32r for continuous-input layers (q,k,v,fc1), fp16 for
  binary-input layers (p, fc2, conv) and attention.
- fc1 z tiles (f16) stay resident in SBUF (no DRAM spill); gated tiles are
  also SBUF-resident through fc2.
"""
import sys
sys.path.insert(0, '/opt/trn_rl_repo')
import numpy as np

import concourse.bass as bass
import concourse.mybir as mybir
import concourse.tile as tile
from concourse.tile import add_dep_helper

T, B, N, C = 4, 128, 64, 384
HID, CH, HEADS, HD = 1536, 768, 12, 32
NCORES = 8
BS = B // NCORES
R = T * BS * N              # 4096 rows per core
TC = BS * N                 # 1024 cols per timestep
COUNT = T * B * N           # 32768 rows globally (BN stat count)
EPS = 1e-5
PADW = 12
PADP = PADW * 10            # 120 per frame plane
NFR = T * BS                # 64 frames
GUARD = 16
PLANE = NFR * PADP          # 7680
PADL = GUARD + PLANE + GUARD

F32 = mybir.dt.float32
F32R = mybir.dt.float32r
F16 = mybir.dt.float16
ALU = mybir.AluOpType
ACTF = mybir.ActivationFunctionType

_ctr = [0]


def _fix_multiwaits(nc):
    """walrus here accepts max 1 sync-wait per instruction: split extras
    onto same-engine NOPs."""
    for f in nc.m.functions:
        for bb in f.blocks:
            new_insts = []
            for inst in bb.instructions:
                si = inst.sync_info
                ow = list(si.on_wait) if (si and si.on_wait) else []
                if len(ow) > 1:
                    for w in ow[:-1]:
                        _ctr[0] += 1
                        new_insts.append(mybir.InstNoOp(
                            name=f"I-waitnop-{_ctr[0]}", engine=inst.engine,
                            sync_info=mybir.SyncInfo(on_wait=[w], on_update=[]),
                            bass_nofuse=True))
                    si.on_wait = [ow[-1]]
                new_insts.append(inst)
            bb.instructions[:] = new_insts


def build_kernel(debug_taps=False, timing=False, stop_after=None):
    nc = bass.Bass("TRN2", target_bir_lowering=False, debug=False,
                   num_devices=NCORES)

    xT_in = nc.declare_dram_parameter("xT", [C, R], F32R, isOutput=False)
    w_in = {}
    for name, ci, co, dt in [("q", C, C, F32R), ("k", C, C, F32R),
                             ("v", C, C, F32R), ("p", C, C, F16),
                             ("fc1", C, HID, F32R), ("fc2", CH, C, F16)]:
        w_in[name] = nc.declare_dram_parameter(f"w_{name}", [ci, co], dt,
                                               isOutput=False)
    pv_in = {}
    for name, co in [("q", C), ("k", C), ("v", C), ("p", C),
                     ("fc1", HID), ("dw", CH), ("fc2", C)]:
        pv_in[name] = nc.declare_dram_parameter(f"pv_{name}", [co, 2], F32,
                                                isOutput=False)
    ident_in = nc.declare_dram_parameter("ident", [128, 128], F16, isOutput=False)
    mask_in = nc.declare_dram_parameter("mask", [128, 512], F16, isOutput=False)
    # 54 diagonal [128,128] f16 weight matrices: (tile i, tap k) at row
    # (i*9+k)*128
    convd_in = nc.declare_dram_parameter("convd", [54 * 128, 128], F16,
                                         isOutput=False)
    if timing:
        out_d = nc.dram_tensor("out", [C, R], F32)
        tok_d = nc.declare_dram_parameter("tok", [128, 1], F32, isOutput=True)
    else:
        out_d = nc.declare_dram_parameter("out", [C, R], F32, isOutput=True)
        tok_d = None

    dbg = {}
    if debug_taps:
        for nm, npt, dt in [("z_q", 3, F16), ("s_q", 3, F16), ("s_k", 3, F16),
                            ("s_v", 3, F16), ("z_y", 3, F16), ("s_y", 3, F16),
                            ("z_p", 3, F16), ("xmid", 3, F32),
                            ("z_fc1", 12, F16), ("z_conv", 6, F16),
                            ("s_conv", 6, F16), ("gated", 6, F16),
                            ("z_fc2", 3, F16)]:
            dbg[nm] = nc.declare_dram_parameter(f"dbg_{nm}", [npt * 128, R],
                                                dt, isOutput=True)

    cc = {}
    for name, co in [("q", C), ("k", C), ("v", C), ("p", C),
                     ("fc1", HID), ("dw", CH), ("fc2", C)]:
        cci = nc.dram_tensor(f"cci_{name}", [co, 2], F32)
        cco = nc.dram_tensor(f"cco_{name}", [co, 2], F32, addr_space="Shared")
        cc[name] = (cci, cco)

    xmid_sp = nc.dram_tensor("xmid_spill", [C, R], F32)
    zx2_sp = nc.dram_tensor("zx2_spill", [CH, R], F16)

    with tile.TileContext(nc, pool_alloc_mode="queue") as tc:
        _body(nc, tc, xT_in, w_in, pv_in, ident_in, mask_in, convd_in, out_d,
              tok_d, cc, xmid_sp, zx2_sp, dbg, stop_after)
    _fix_multiwaits(nc)
    return nc


def _body(nc, tc, xT_in, w_in, pv_in, ident_in, mask_in, convd_in, out_d,
          tok_d, cc, xmid_sp, zx2_sp, dbg, stop_after=None):
    from contextlib import ExitStack

    # spike DRAM buffers (cross-phase hand-off)
    s_d = {name: nc.dram_tensor(f"s{name}_d", [C, R], F16)
           for name in ("q", "k", "v", "y")}

    # ---------- long-lived small pools ----------
    ctxL = ExitStack()
    const_p = ctxL.enter_context(tc.tile_pool(name="const", bufs=1))
    stat_p = ctxL.enter_context(tc.tile_pool(name="stats", bufs=1))
    scr_p = ctxL.enter_context(tc.tile_pool(name="scr", bufs=2))
    lif_p = ctxL.enter_context(tc.tile_pool(name="lifp", bufs=1))
    ps_mm = ctxL.enter_context(tc.tile_pool(name="psmm", bufs=4, space="PSUM"))
    ps_at = ctxL.enter_context(tc.tile_pool(name="psat", bufs=1, space="PSUM"))

    ident = const_p.tile([128, 128], F16, tag="ident", name="ident")
    nc.sync.dma_start(ident[:], ident_in[:])
    mask = const_p.tile([128, 512], F16, tag="mask", name="mask")
    nc.sync.dma_start(mask[:], mask_in[:])
    pvec = {}
    for name, npt in [("q", 3), ("k", 3), ("v", 3), ("p", 3),
                      ("fc1", 12), ("dw", 6), ("fc2", 3)]:
        pv = const_p.tile([128, 2 * npt], F32, tag=f"pv_{name}",
                          name=f"pv_{name}")
        for i in range(npt):
            nc.sync.dma_start(pv[:, 2 * i:2 * i + 2],
                              pv_in[name][128 * i:128 * (i + 1), :])
        pvec[name] = pv

    # stats: sump, sqp [128, ngrp*npt]; stfin [128, 2*npt] (S,Q); AC [128,2*npt]
    NGRP = {"q": 8, "k": 8, "v": 8, "p": 8, "fc1": 8, "fc2": 8, "dw": 1}
    STT = {}
    for name, npt in [("q", 3), ("k", 3), ("v", 3), ("p", 3),
                      ("fc1", 12), ("dw", 6), ("fc2", 3)]:
        g = NGRP[name]
        STT[name] = (
            stat_p.tile([128, g * npt], F32, tag=f"sum_{name}", name=f"sum_{name}"),
            stat_p.tile([128, g * npt], F32, tag=f"sq_{name}", name=f"sq_{name}"),
            stat_p.tile([128, 2 * npt], F32, tag=f"st_{name}", name=f"st_{name}"),
            stat_p.tile([128, 2 * npt], F32, tag=f"ac_{name}", name=f"ac_{name}"),
        )

    # ---------- helpers ----------
    def emit_linear(name, wt, rhs, co_lo, co_hi, n_ci, z_alloc, z_done):
        """z tiles are f16. PSUM evac on ACT (with sum accum); sq-sums on DVE
        via tensor_tensor_reduce."""
        sump, sqp = STT[name][0], STT[name][1]
        for co in range(co_lo, co_hi):
            z = z_alloc(co)
            for cg in range(2):
                pss = [ps_mm.tile([128, 512], F32, tag="ps", name=f"ps{name}{co}{cg}{j}")
                       for j in range(4)]
                for ci in range(n_ci):
                    for ch in range(4):
                        nc.tensor.matmul(
                            pss[ch][:],
                            lhsT=wt[ci][:, co * 128:(co + 1) * 128],
                            rhs=rhs[ci][:, (cg * 4 + ch) * 512:(cg * 4 + ch + 1) * 512],
                            start=(ci == 0), stop=(ci == n_ci - 1))
                for ch in range(4):
                    g = cg * 4 + ch
                    sl = slice(g * 512, (g + 1) * 512)
                    nc.scalar.activation(z[:, sl], pss[ch][:], ACTF.Copy,
                                         accum_out=sump[:, co * 8 + g:co * 8 + g + 1])
                    sq = scr_p.tile([128, 512], F16, tag="sqscr", name="sqscr")
                    if g % 2 == 0:
                        nc.vector.scalar_tensor_tensor(
                            sq[:], z[:, sl], 1.0, z[:, sl], ALU.mult, ALU.mult,
                            accum_out=sqp[:, co * 8 + g:co * 8 + g + 1])
                    else:
                        nc.scalar.activation(
                            sq[:], pss[ch][:], ACTF.Square,
                            accum_out=sqp[:, co * 8 + g:co * 8 + g + 1])
            z_done(co, z)

    def emit_ar(name, lo, hi):
        """AllReduce stats for ptiles [lo, hi) of `name`."""
        ngrp = NGRP[name]
        sump, sqp, stfin, _ = STT[name]
        for co in range(lo, hi):
            if ngrp == 1:
                nc.vector.tensor_copy(stfin[:, 2 * co:2 * co + 1],
                                      sump[:, co:co + 1])
                nc.vector.tensor_copy(stfin[:, 2 * co + 1:2 * co + 2],
                                      sqp[:, co:co + 1])
            else:
                nc.vector.tensor_reduce(stfin[:, 2 * co:2 * co + 1],
                                        sump[:, co * ngrp:(co + 1) * ngrp],
                                        axis=mybir.AxisListType.X, op=ALU.add)
                nc.vector.tensor_reduce(stfin[:, 2 * co + 1:2 * co + 2],
                                        sqp[:, co * ngrp:(co + 1) * ngrp],
                                        axis=mybir.AxisListType.X, op=ALU.add)
        cci, cco = cc[name]
        dmas = []
        for co in range(lo, hi):
            d = nc.sync.dma_start(cci[128 * co:128 * (co + 1), :],
                                  stfin[:, 2 * co:2 * co + 2])
            dmas.append(d)
        ar = nc.gpsimd.collective_compute(
            "AllReduce", ALU.add, replica_groups=[list(range(NCORES))],
            ins=[cci[128 * lo:128 * hi, :]], outs=[cco[128 * lo:128 * hi, :]])
        for d in dmas:
            add_dep_helper(ar.ins, d.ins, reason="ar waits dma_in")
        for co in range(lo, hi):
            d = nc.sync.dma_start(stfin[:, 2 * co:2 * co + 2],
                                  cco[128 * co:128 * (co + 1), :])
            add_dep_helper(d.ins, ar.ins, reason="readback waits ar")

    def emit_params(name, lo, hi):
        """Per ptile compute A = 0.5*a, C = 0.5*c into AC. All ops on [128,1]
        tiles with tensor_tensor / immediate tensor_scalar (fast paths)."""
        _, _, stfin, AC = STT[name]
        pv = pvec[name]
        for i in range(lo, hi):
            S_ = stfin[:, 2 * i:2 * i + 1]
            Q_ = stfin[:, 2 * i + 1:2 * i + 2]
            g_ = pv[:, 2 * i:2 * i + 1]
            be_ = pv[:, 2 * i + 1:2 * i + 2]
            A_ = AC[:, 2 * i:2 * i + 1]
            C_ = AC[:, 2 * i + 1:2 * i + 2]
            w = scr_p.tile([128, 6], F32, tag="pscr", name="pscr")
            mean, qm, var, sd, inv, a_ = (w[:, j:j + 1] for j in range(6))
            nc.vector.tensor_scalar(mean, S_, 1.0 / COUNT, None, ALU.mult)
            nc.vector.tensor_scalar(qm, Q_, 1.0 / COUNT, None, ALU.mult)
            nc.vector.tensor_tensor(var, mean, mean, ALU.mult)
            nc.vector.tensor_tensor(var, qm, var, ALU.subtract)
            nc.vector.tensor_scalar(var, var, EPS, None, ALU.add)
            nc.scalar.sqrt(sd, var)
            nc.vector.reciprocal(inv, sd)
            nc.vector.tensor_tensor(a_, g_, inv, ALU.mult)
            nc.vector.tensor_scalar(A_, a_, 0.5, None, ALU.mult)
            # C = 0.5*(be - mean*a)
            nc.vector.tensor_tensor(C_, mean, a_, ALU.mult)
            nc.vector.tensor_tensor(C_, be_, C_, ALU.subtract)
            nc.vector.tensor_scalar(C_, C_, 0.5, None, ALU.mult)

    def emit_norm(name, z, yh, pt_off):
        """yh = 0.5*a*z + 0.5*c on ACT (per-partition affine), f16 out."""
        AC = STT[name][3]
        nc.scalar.activation(yh[:], z[:], ACTF.Identity,
                             scale=AC[:, 2 * pt_off:2 * pt_off + 1],
                             bias=AC[:, 2 * pt_off + 1:2 * pt_off + 2])

    def emit_lif(yh, writer, thr=1.0, sdt=F16):
        """LIF over yh [128, R] (already includes the 0.5 input scale).
        u_t = 0.5*u_{t-1}*[u_{t-1} < thr] + yh_t ; writer(t, u) emits spikes.
        All immediate-scalar TS/TT ops."""
        tg = "32" if sdt == F32 else "16"
        u = lif_p.tile([128, TC], sdt, tag="lifu" + tg, name="lifu", bufs=2)
        sc = lif_p.tile([128, TC], sdt, tag="lifsc" + tg, name="lifsc", bufs=2)
        h = lif_p.tile([128, TC], sdt, tag="lifh" + tg, name="lifh", bufs=2)
        for t in range(T):
            ucur = yh[:, 0:TC] if t == 0 else u[:]
            writer(t, ucur)
            if t < T - 1:
                nc.vector.tensor_scalar(sc[:], ucur, thr, 0.5,
                                        ALU.is_lt, ALU.mult)
                nc.vector.tensor_tensor(h[:], ucur, sc[:], ALU.mult)
                nc.vector.tensor_tensor(u[:], h[:],
                                        yh[:, (t + 1) * TC:(t + 2) * TC],
                                        ALU.add)

    def spike_writer(st, thr=1.0, eng=None):
        e = eng if eng is not None else nc.vector
        def w(t, ucur, st=st, thr=thr, e=e):
            e.tensor_scalar(st[:, t * TC:(t + 1) * TC], ucur, thr, None,
                            ALU.is_ge)
        return w

    def dump_rows(nm, row0, t_):
        if nm in dbg:
            nc.sync.dma_start(dbg[nm][row0:row0 + 128, :], t_[:])

    # ============ PHASE 1: q,k,v matmul + AR + LIF -> spikes to DRAM ======
    ctxA = ExitStack()
    pA = ctxA.enter_context(tc.tile_pool(name="pA", bufs=1))
    xT = []
    for i in range(3):
        x = pA.tile([128, R], F32R, tag=f"xT{i}", name=f"xT{i}")
        nc.sync.dma_start(x[:], xT_in[128 * i:128 * (i + 1), :])
        xT.append(x)
    wts = {}
    for name in ("q", "k", "v"):
        wt = []
        for i in range(3):
            w = pA.tile([128, C], F32R, tag=f"w_{name}{i}", name=f"w_{name}{i}")
            nc.sync.dma_start(w[:], w_in[name][128 * i:128 * (i + 1), :])
            wt.append(w)
        wts[name] = wt

    zs = {}
    for name in ("q", "k", "v"):
        zt = []

        def zalloc(co, name=name, zt=zt):
            z = pA.tile([128, R], F16, tag=f"z{co}", name=f"z{name}{co}", bufs=2)
            zt.append(z)
            return z

        emit_linear(name, wts[name], xT, 0, 3, 3, zalloc, lambda co, z: None)
        zs[name] = zt
        emit_ar(name, 0, 3)
        emit_params(name, 0, 3)
    for pt in range(3):
        dump_rows("z_q", 128 * pt, zs["q"][pt])

    for name in ("q", "k", "v"):
        for pt in range(3):
            yh = pA.tile([128, R], F32, tag=f"yh{pt % 2}", name=f"yh{name}{pt}")
            emit_norm(name, zs[name][pt], yh, pt)
            st = pA.tile([128, R], F16, tag=f"spt{pt % 2}", name=f"s{name}{pt}")
            emit_lif(yh, spike_writer(st), sdt=F32)
            nc.sync.dma_start(s_d[name][128 * pt:128 * (pt + 1), :], st[:])
            dump_rows(f"s_{name}", 128 * pt, st)
    ctxA.close()
    if stop_after == 'qkv':
        ctxL.close(); return

    # ============ PHASE 2: transposes + attention + y-LIF ============
    ctxB = ExitStack()
    pB = ctxB.enter_context(tc.tile_pool(name="pB", bufs=1))
    # rm layout: per (pt, tb) a (128, 128) block at col (pt*64+tb)*128;
    # rows 0..63 = transposed spikes (n-major), rows 64..127 stay ZERO so
    # mm1 can contract over the full K=128 (K=64 matmuls hang on this HW).
    rm = {}
    for name in ("k", "v"):
        rmt = pB.tile([128, 6 * R], F16, tag=f"rm_{name}", name=f"rm_{name}")
        nc.gpsimd.memset(rmt[64:128, :], 0.0)
        for pt in range(3):
            srt = pB.tile([128, R], F16, tag=f"skvr{pt % 2}",
                          name=f"r{name}{pt}")
            nc.sync.dma_start(srt[:], s_d[name][128 * pt:128 * (pt + 1), :])
            for grp in range(16):
                pst = ps_at.tile([128, 512], F16, tag="pstr", name="pstr")
                for j in range(4):
                    tb = grp * 4 + j
                    nc.tensor.transpose(pst[0:64, 128 * j:128 * (j + 1)],
                                        srt[:, 64 * tb:64 * (tb + 1)],
                                        ident[:])
                nc.scalar.copy(
                    rmt[0:64, (pt * 64 + grp * 4) * 128:(pt * 64 + grp * 4 + 4) * 128],
                    pst[0:64, :])
        rm[name] = rmt

    sy = []
    for pt in range(3):
        sqr = pB.tile([128, R], F16, tag=f"sqr{pt % 2}", name=f"sqr{pt}")
        nc.sync.dma_start(sqr[:], s_d["q"][128 * pt:128 * (pt + 1), :])
        zy = pB.tile([128, R], F16, tag=f"zy{pt % 2}", name=f"zy{pt}")
        for g4 in range(16):
            mm1ps = ps_at.tile([128, 512], F32, tag="mm1", name="mm1")
            for j in range(4):
                tb = g4 * 4 + j
                base = (pt * 64 + tb) * 128
                nc.tensor.matmul(mm1ps[:, 128 * j:128 * (j + 1)],
                                 lhsT=rm["k"][:, base:base + 128],
                                 rhs=rm["v"][:, base:base + 128],
                                 start=True, stop=True)
            m4 = scr_p.tile([128, 512], F16, tag="m4", name="m4")
            nc.vector.tensor_tensor(m4[:], mm1ps[:], mask[:], ALU.mult)
            yps = ps_at.tile([128, 256], F32, tag="yps", name="yps", bufs=2)
            for j in range(4):
                tb = g4 * 4 + j
                nc.tensor.matmul(yps[:, 64 * j:64 * (j + 1)],
                                 lhsT=m4[:, 128 * j:128 * (j + 1)],
                                 rhs=sqr[:, 64 * tb:64 * (tb + 1)],
                                 start=True, stop=True)
            # evacuate with 0.5 scale: zy holds Y = 0.5 * z_y
            nc.scalar.activation(zy[:, 256 * g4:256 * (g4 + 1)], yps[:],
                                 ACTF.Copy, scale=0.5)
        dump_rows("z_y", 128 * pt, zy)
        syt = pB.tile([128, R], F16, tag=f"sy{pt % 2}", name=f"sy{pt}")
        emit_lif(zy, spike_writer(syt, thr=0.5), thr=0.5)
        nc.sync.dma_start(s_d["y"][128 * pt:128 * (pt + 1), :], syt[:])
        dump_rows("s_y", 128 * pt, syt)
    ctxB.close()
    if stop_after == 'attn':
        ctxL.close(); return

    # ============ PHASE 3: p-linear + xmid ============
    ctxC = ExitStack()
    pC = ctxC.enter_context(tc.tile_pool(name="pC", bufs=1))
    syr = []
    for i in range(3):
        s = pC.tile([128, R], F16, tag=f"syr{i}", name=f"syr{i}")
        nc.sync.dma_start(s[:], s_d["y"][128 * i:128 * (i + 1), :])
        syr.append(s)
    wt_p = []
    for i in range(3):
        w = pC.tile([128, C], F16, tag=f"w_p{i}", name=f"w_p{i}")
        nc.sync.dma_start(w[:], w_in["p"][128 * i:128 * (i + 1), :])
        wt_p.append(w)
    zp = []

    def zalloc_p(co):
        z = pC.tile([128, R], F16, tag=f"zp{co}", name=f"zp{co}")
        zp.append(z)
        return z

    emit_linear("p", wt_p, syr, 0, 3, 3, zalloc_p, lambda co, z: None)
    emit_ar("p", 0, 3)
    emit_params("p", 0, 3)
    for pt in range(3):
        dump_rows("z_p", 128 * pt, zp[pt])
        yh = pC.tile([128, R], F32, tag="yhp", name=f"yhp{pt}", bufs=2)
        emit_norm("p", zp[pt], yh, pt)
        spt = pC.tile([128, R], F16, tag="sptr", name=f"sp{pt}", bufs=2)
        emit_lif(yh, spike_writer(spt), sdt=F32)
        # xr = x + p_spikes ; spill to DRAM (read back in phases 4 and 6)
        xr = pC.tile([128, R], F32, tag=f"xm{pt % 2}", name=f"xm{pt}", bufs=2)
        nc.sync.dma_start(xr[:].bitcast(F32R), xT_in[128 * pt:128 * (pt + 1), :])
        nc.vector.tensor_tensor(xr[:], xr[:], spt[:], ALU.add)
        nc.sync.dma_start(xmid_sp[128 * pt:128 * (pt + 1), :], xr[:])
        dump_rows("xmid", 128 * pt, xr)
    ctxC.close()
    if stop_after == 'p':
        ctxL.close(); return

    # ============ PHASE 4: fc1 (z tiles stay in SBUF) ============
    ctxZ = ExitStack()
    pZ = ctxZ.enter_context(tc.tile_pool(name="pZ", bufs=1))
    ctxD = ExitStack()
    pD = ctxD.enter_context(tc.tile_pool(name="pD", bufs=1))
    wt_fc1 = []
    for i in range(3):
        w = pD.tile([128, HID], F32R, tag=f"wfc1_{i}", name=f"wfc1_{i}")
        nc.sync.dma_start(w[:], w_in["fc1"][128 * i:128 * (i + 1), :])
        wt_fc1.append(w)
    xmid_v = []
    for i in range(3):
        x = pD.tile([128, R], F32R, tag=f"xmid{i}", name=f"xmid{i}")
        nc.sync.dma_start(x[:].bitcast(F32), xmid_sp[128 * i:128 * (i + 1), :])
        xmid_v.append(x)

    zf1 = {}

    def zalloc_f(co):
        if co < 6:
            z = pZ.tile([128, R], F16, tag=f"zf1_{co}", name=f"zf1_{co}")
            zf1[co] = z
        else:
            z = pD.tile([128, R], F16, tag=f"zx2_{co % 2}", name=f"zf1_{co}",
                        bufs=2)
        return z

    def zdone_f(co, z):
        if co >= 6:
            nc.sync.dma_start(zx2_sp[128 * (co - 6):128 * (co - 5), :], z[:])
        if "z_fc1" in dbg:
            nc.sync.dma_start(dbg["z_fc1"][128 * co:128 * (co + 1), :], z[:])

    emit_linear("fc1", wt_fc1, xmid_v, 0, 12, 3, zalloc_f, zdone_f)
    emit_ar("fc1", 0, 12)
    emit_params("fc1", 0, 12)
    ctxD.close()
    if stop_after == 'fc1':
        ctxZ.close(); ctxL.close(); return

    # ============ PHASE 5a: x1-LIF -> spike planes -> PE conv ============
    ctxE = ExitStack()
    pE = ctxE.enter_context(tc.tile_pool(name="pE", bufs=1))
    convd = pE.tile([128, 54 * 128], F16, tag="convd", name="convd")
    for i in range(54):
        nc.sync.dma_start(convd[:, 128 * i:128 * (i + 1)],
                          convd_in[128 * i:128 * (i + 1), :])
    # tap shift offsets in plane space, kh-major to match host convd order
    SHIFTS = [dh * PADW + dw for dh in (-1, 0, 1) for dw in (-1, 0, 1)]

    z_conv = []
    sump_c, sqp_c, _, _ = STT["dw"]
    for i in range(6):
        yh = pE.tile([128, R], F32, tag="yhc", name=f"yhx1_{i}")
        emit_norm("fc1", zf1[i], yh, i)
        xa = pE.tile([128, PADL], F16, tag=f"cxa{i % 2}", name=f"cxa{i}")
        nc.gpsimd.memset(xa[:], 0.0)

        def x1_writer(t, ucur, xa=xa):
            # one strided is_ge into the padded plane per timestep
            xa4 = xa[:, GUARD + t * BS * PADP:GUARD + (t + 1) * BS * PADP] \
                .rearrange("p (f r w) -> p f r w", r=10, w=PADW)[:, :, 1:9, 2:10]
            u4 = ucur.rearrange("p (f h w) -> p f h w", h=8, w=8)
            nc.vector.tensor_scalar(xa4, u4, 1.0, None, ALU.is_ge)

        emit_lif(yh, x1_writer, sdt=F32)

        # 9-tap depthwise conv via diagonal-weight matmuls, plane space
        zpl = pE.tile([128, PLANE], F16, tag="zpl", name=f"zpl{i}")
        for c in range(15):
            cps = ps_mm.tile([128, 512], F32, tag="ps", name=f"cps{i}{c}")
            for k in range(9):
                base = GUARD + c * 512 + SHIFTS[k]
                nc.tensor.matmul(cps[:],
                                 lhsT=convd[:, (i * 9 + k) * 128:(i * 9 + k + 1) * 128],
                                 rhs=xa[:, base:base + 512],
                                 start=(k == 0), stop=(k == 8))
            nc.scalar.copy(zpl[:, 512 * c:512 * (c + 1)], cps[:])
        # repack valid positions (f,1..8,2..9) -> contiguous z_conv + stats
        zc = pZ.tile([128, R], F16, tag=f"zf1_{i}", name=f"zconv{i}")
        zpl4 = zpl[:].rearrange("p (f r w) -> p f r w", r=10, w=PADW)[:, :, 1:9, 2:10]
        nc.scalar.activation(zc[:].rearrange("p (f h w) -> p f h w", h=8, w=8),
                             zpl4, ACTF.Copy,
                             accum_out=sump_c[:, i:i + 1])
        sq = pE.tile([128, R], F16, tag="sqcv", name=f"sqc{i}")
        nc.vector.scalar_tensor_tensor(
            sq[:], zc[:], 1.0, zc[:], ALU.mult, ALU.mult,
            accum_out=sqp_c[:, i:i + 1])
        z_conv.append(zc)
    emit_ar("dw", 0, 6)
    emit_params("dw", 0, 6)
    for i in range(6):
        dump_rows("z_conv", 128 * i, z_conv[i])

    # ============ PHASE 5b: x2-LIF + conv-LIF + gating ============
    gated = []
    for i in range(6):
        zx2 = pE.tile([128, R], F16, tag="zx2r", name=f"zx2r{i}")
        nc.sync.dma_start(zx2[:], zx2_sp[128 * i:128 * (i + 1), :])
        yh2 = pE.tile([128, R], F32, tag="yhc", name=f"yhx2_{i}")
        emit_norm("fc1", zx2, yh2, 6 + i)
        sx2 = pE.tile([128, R], F16, tag="gt", name=f"sx2_{i}")
        emit_lif(yh2, spike_writer(sx2), sdt=F32)
        yhc = pE.tile([128, R], F16, tag="yhcv", name=f"yhcv{i}")
        emit_norm("dw", z_conv[i], yhc, i)
        scv = pE.tile([128, R], F16, tag="scv", name=f"scv{i}")
        emit_lif(yhc, spike_writer(scv))
        dump_rows("s_conv", 128 * i, scv)
        g = pZ.tile([128, R], F16, tag=f"zf1_{i}", name=f"gated{i}")
        nc.vector.tensor_tensor(g[:], scv[:], sx2[:], ALU.mult)
        gated.append(g)
        dump_rows("gated", 128 * i, g)
    ctxE.close()
    if stop_after == 'conv':
        ctxZ.close(); ctxL.close(); return

    # ============ PHASE 6: fc2 + final residual ============
    ctxG = ExitStack()
    pG = ctxG.enter_context(tc.tile_pool(name="pG", bufs=1))
    wt_fc2 = []
    for i in range(6):
        w = pG.tile([128, C], F16, tag=f"wfc2_{i}", name=f"wfc2_{i}")
        nc.sync.dma_start(w[:], w_in["fc2"][128 * i:128 * (i + 1), :])
        wt_fc2.append(w)
    zf2 = []

    def zalloc_g(co):
        z = pG.tile([128, R], F16, tag=f"zf2{co}", name=f"zf2{co}")
        zf2.append(z)
        return z

    emit_linear("fc2", wt_fc2, gated, 0, 3, 6, zalloc_g, lambda co, z: None)
    emit_ar("fc2", 0, 3)
    emit_params("fc2", 0, 3)
    for pt in range(3):
        dump_rows("z_fc2", 128 * pt, zf2[pt])
        yh = pG.tile([128, R], F32, tag="yhg", name=f"yhg{pt}", bufs=2)
        emit_norm("fc2", zf2[pt], yh, pt)
        so = pG.tile([128, R], F16, tag="so", name=f"so{pt}", bufs=2)
        emit_lif(yh, spike_writer(so), sdt=F32)
        xm = pG.tile([128, R], F32, tag=f"xmr{pt % 2}", name=f"xmr{pt}")
        nc.sync.dma_start(xm[:], xmid_sp[128 * pt:128 * (pt + 1), :])
        nc.vector.tensor_tensor(xm[:], xm[:], so[:], ALU.add)
        nc.sync.dma_start(out_d[128 * pt:128 * (pt + 1), :], xm[:])
    if tok_d is not None:
        tk = pG.tile([128, 1], F32, tag="tok", name="tk")
        nc.vector.memset(tk[:], 1.0)
        nc.sync.dma_start(tok_d[:], tk[:])
    ctxG.close()
    ctxZ.close()
    ctxL.close()


# ---------------- host glue ----------------

def _prep_inputs(inputs):
    x = np.asarray(inputs['x'], np.float32)
    xr = x.reshape(T, B, N, C)
    ident = np.eye(128, dtype=np.float16)
    mask = np.zeros((128, 512), np.float16)
    for blk in range(4):
        for h in range(4):
            mask[h * 32:(h + 1) * 32,
                 blk * 128 + h * 32:blk * 128 + (h + 1) * 32] = 0.125
    common = {"ident": ident, "mask": mask}
    for name in ("q", "k", "v", "p", "fc1", "fc2"):
        wdt = np.float16 if name in ("p", "fc2") else np.float32
        common[f"w_{name}"] = np.ascontiguousarray(
            np.asarray(inputs[name + "_w"]).T).astype(wdt)
    for name in ("q", "k", "v", "p", "fc1", "fc2"):
        common[f"pv_{name}"] = np.ascontiguousarray(np.stack(
            [np.asarray(inputs[name + "_g"], np.float32),
             np.asarray(inputs[name + "_be"], np.float32)], 1))
    common["pv_dw"] = np.ascontiguousarray(np.stack(
        [np.asarray(inputs["dw_g"], np.float32),
         np.asarray(inputs["dw_be"], np.float32)], 1))
    kv = np.asarray(inputs["dw_k"], np.float32).reshape(CH, 9)
    convd = np.zeros((54 * 128, 128), np.float16)
    for i in range(6):
        for k in range(9):
            blk = np.diag(kv[i * 128:(i + 1) * 128, k]).astype(np.float16)
            convd[(i * 9 + k) * 128:(i * 9 + k + 1) * 128, :] = blk
    common["convd"] = convd

    maps = []
    for c in range(NCORES):
        shard = xr[:, c * BS:(c + 1) * BS]
        xt = np.ascontiguousarray(shard.reshape(R, C).T)
        m = dict(common)
        m["xT"] = xt
        maps.append(m)
    return maps


_CACHE = {}


def _get_runner(debug_taps=False, timing=False, stop_after=None):
    key = (debug_taps, timing, stop_after)
    if key not in _CACHE:
        from runner_embed import SpmdRunner
        nc = build_kernel(debug_taps, timing, stop_after)
        _CACHE[key] = SpmdRunner(nc, NCORES)
    return _CACHE[key]


def kernel(**inputs):
    r = _get_runner()
    maps = _prep_inputs(inputs)
    args = r.prep(maps)
    outs = r.run(args)
    res = r.results(outs)
    full = np.empty((T, B, N, C), np.float32)
    for c in range(NCORES):
        o = res[c]["out"]
        full[:, c * BS:(c + 1) * BS] = o.T.reshape(T, BS, N, C)
    return np.ascontiguousarray(full.reshape(T * B, N, C))


# ---- embedded SPMD runner module ----
import types
runner_embed = types.ModuleType("runner_embed")
sys.modules["runner_embed"] = runner_embed
exec(r'''
import sys
sys.path.insert(0, '/opt/trn_rl_repo')
import numpy as np
import jax
from jax.sharding import Mesh, PartitionSpec
from jax.experimental.shard_map import shard_map
import concourse.bass as bass
import concourse.mybir as mybir
from concourse.bass2jax import _bass_exec_p, install_neuronx_cc_hook, partition_id_tensor


class SpmdRunner:
    def __init__(self, nc, n_cores):
        install_neuronx_cc_hook()
        self.nc = nc
        self.n_cores = n_cores
        partition_name = nc.partition_id_tensor.name if nc.partition_id_tensor else None
        in_names, out_names, out_avals, zero_outs = [], [], [], []
        for alloc in nc.m.functions[0].allocations:
            if not isinstance(alloc, mybir.MemoryLocationSet):
                continue
            name = alloc.memorylocations[0].name
            if alloc.kind == "ExternalInput":
                if name != partition_name:
                    in_names.append(name)
            elif alloc.kind == "ExternalOutput":
                shape = tuple(alloc.tensor_shape)
                dtype = mybir.dt.np(alloc.dtype)
                out_names.append(name)
                out_avals.append(jax.core.ShapedArray(shape, dtype))
                zero_outs.append(np.zeros(shape, dtype))
        self.in_names, self.out_names = in_names, out_names
        self.out_avals, self.zero_outs = out_avals, zero_outs
        n_params = len(in_names)
        n_outs = len(out_avals)
        all_in_names = list(in_names) + list(out_names)
        if partition_name is not None:
            all_in_names.append(partition_name)
        self.n_params = n_params

        def _body(*args):
            operands = list(args)
            if partition_name is not None:
                operands.append(partition_id_tensor())
            outs = _bass_exec_p.bind(
                *operands, out_avals=tuple(out_avals),
                in_names=tuple(all_in_names), out_names=tuple(out_names),
                lowering_input_output_aliases=(),
                sim_require_finite=True, sim_require_nnan=True, nc=nc)
            return tuple(outs)

        devices = jax.devices()[:n_cores]
        assert len(devices) == n_cores
        mesh = Mesh(np.asarray(devices), ("core",))
        in_specs = (PartitionSpec("core"),) * (n_params + n_outs)
        out_specs = (PartitionSpec("core"),) * n_outs
        self.fn = jax.jit(
            shard_map(_body, mesh=mesh, in_specs=in_specs,
                      out_specs=out_specs, check_rep=False),
            keep_unused=True)

    def prep(self, in_maps):
        per_core = [[np.asarray(m[name]) for name in self.in_names]
                    for m in in_maps]
        concat_in = [np.concatenate([per_core[c][i] for c in range(self.n_cores)], axis=0)
                     for i in range(self.n_params)]
        concat_zeros = [np.zeros((self.n_cores * z.shape[0], *z.shape[1:]), z.dtype)
                        for z in self.zero_outs]
        return [jax.device_put(a) for a in concat_in + concat_zeros]

    def run(self, args):
        outs = self.fn(*args)
        jax.block_until_ready(outs)
        return outs

    def results(self, outs):
        res = []
        for c in range(self.n_cores):
            res.append({name: np.asarray(outs[i]).reshape(self.n_cores, *self.out_avals[i].shape)[c]
                        for i, name in enumerate(self.out_names)})
        return res

    def time_it(self, args, iters=20, warmup=3):
        import time
        for _ in range(warmup):
            self.run(args)
        ts = []
        for _ in range(iters):
            t0 = time.perf_counter()
            self.run(args)
            ts.append(time.perf_counter() - t0)
        ts = np.array(ts)
        return dict(min=ts.min(), median=float(np.median(ts)), mean=ts.mean())
''', runner_embed.__dict__)



# revision 4
# speedup vs baseline: 9.8102x; 9.8102x over previous
"""Trainium2 Bass kernel for nn_Block_76519137345684 (Spikformer-style block:
spiking self-attention + spiking gated MLP with training-mode BatchNorm).

Strategy
- Data-parallel over batch B across 8 NeuronCores (16 batch each). BN batch
  statistics (per-channel sum / sum-of-squares) are AllReduced across cores.
- Activations live channel-on-partition: (C, rows) with rows
  r = ((t*16 + b)*64 + n); LIF timesteps are contiguous 1024-column slices.
- BN is applied as ONE ACT-engine pass per ptile after the stats AllReduce:
  yh = 0.5*a*z + 0.5*c  (a = g*rsqrt(var+eps), c = be - mu*a), f16 output.
  The LIF recurrence then has UNIFORM immediate thresholds:
      u_t = 0.5*u_{t-1}*[u_{t-1} < thr] + yh_t,   s_t = [u_t >= thr]
  implemented with tensor_scalar/tensor_tensor ops only (no per-partition
  scalar operands, no scalar_tensor_tensor) - those are the fast DVE paths.
- Attention uses associativity: y = q @ (k^T v) * scale; per-head block
  structure enforced with a 0.125-scaled block-diagonal mask.
- Depthwise 3x3 conv runs on the TENSOR engine: spikes are written into a
  zero-padded plane layout (10 rows x 12 cols per frame); 9 taps become 9
  PSUM-accumulated matmuls with diagonal per-channel weight matrices against
  shifted views of the plane. Valid positions are repacked on ACT.
- Matmul dtypes: f32r for continuous-input layers (q,k,v,fc1), fp16 for
  binary-input layers (p, fc2, conv) and attention.
- fc1 z tiles (f16) stay resident in SBUF (no DRAM spill); gated tiles are
  also SBUF-resident through fc2.
"""
import sys
sys.path.insert(0, '/opt/trn_rl_repo')
import numpy as np

import concourse.bass as bass
import concourse.mybir as mybir
import concourse.tile as tile
from concourse.tile import add_dep_helper

T, B, N, C = 4, 128, 64, 384
HID, CH, HEADS, HD = 1536, 768, 12, 32
NCORES = 8
BS = B // NCORES
R = T * BS * N              # 4096 rows per core
TC = BS * N                 # 1024 cols per timestep
COUNT = T * B * N           # 32768 rows globally (BN stat count)
EPS = 1e-5
PADW = 12
PADP = PADW * 10            # 120 per frame plane
NFR = T * BS                # 64 frames
GUARD = 16
PLANE = NFR * PADP          # 7680
PADL = GUARD + PLANE + GUARD

F32 = mybir.dt.float32
F32R = mybir.dt.float32r
F16 = mybir.dt.float16
ALU = mybir.AluOpType
ACTF = mybir.ActivationFunctionType

_ctr = [0]


def _fix_multiwaits(nc):
    """walrus here accepts max 1 sync-wait per instruction: split extras
    onto same-engine NOPs."""
    for f in nc.m.functions:
        for bb in f.blocks:
            new_insts = []
            for inst in bb.instructions:
                si = inst.sync_info
                ow = list(si.on_wait) if (si and si.on_wait) else []
                if len(ow) > 1:
                    for w in ow[:-1]:
                        _ctr[0] += 1
                        new_insts.append(mybir.InstNoOp(
                            name=f"I-waitnop-{_ctr[0]}", engine=inst.engine,
                            sync_info=mybir.SyncInfo(on_wait=[w], on_update=[]),
                            bass_nofuse=True))
                    si.on_wait = [ow[-1]]
                new_insts.append(inst)
            bb.instructions[:] = new_insts


def build_kernel(debug_taps=False, timing=False, stop_after=None):
    nc = bass.Bass("TRN2", target_bir_lowering=False, debug=False,
                   num_devices=NCORES)

    xT_in = nc.declare_dram_parameter("xT", [C, R], F32R, isOutput=False)
    w_in = {}
    for name, ci, co, dt in [("q", C, C, F32R), ("k", C, C, F32R),
                             ("v", C, C, F32R), ("p", C, C, F16),
                             ("fc1", C, HID, F32R), ("fc2", CH, C, F16)]:
        w_in[name] = nc.declare_dram_parameter(f"w_{name}", [ci, co], dt,
                                               isOutput=False)
    pv_in = {}
    for name, co in [("q", C), ("k", C), ("v", C), ("p", C),
                     ("fc1", HID), ("dw", CH), ("fc2", C)]:
        pv_in[name] = nc.declare_dram_parameter(f"pv_{name}", [co, 2], F32,
                                                isOutput=False)
    ident_in = nc.declare_dram_parameter("ident", [128, 128], F16, isOutput=False)
    mask_in = nc.declare_dram_parameter("mask", [128, 512], F16, isOutput=False)
    # 54 diagonal [128,128] f16 weight matrices: (tile i, tap k) at row
    # (i*9+k)*128
    convd_in = nc.declare_dram_parameter("convd", [54 * 128, 128], F16,
                                         isOutput=False)
    if timing:
        out_d = nc.dram_tensor("out", [C, R], F32)
        tok_d = nc.declare_dram_parameter("tok", [128, 1], F32, isOutput=True)
    else:
        out_d = nc.declare_dram_parameter("out", [C, R], F32, isOutput=True)
        tok_d = None

    dbg = {}
    if debug_taps:
        for nm, npt, dt in [("z_q", 3, F16), ("s_q", 3, F16), ("s_k", 3, F16),
                            ("s_v", 3, F16), ("z_y", 3, F16), ("s_y", 3, F16),
                            ("z_p", 3, F16), ("xmid", 3, F32),
                            ("z_fc1", 12, F16), ("z_conv", 6, F16),
                            ("s_conv", 6, F16), ("gated", 6, F16),
                            ("z_fc2", 3, F16)]:
            dbg[nm] = nc.declare_dram_parameter(f"dbg_{nm}", [npt * 128, R],
                                                dt, isOutput=True)

    cc = {}
    for name, co in [("q", C), ("k", C), ("v", C), ("p", C),
                     ("fc1", HID), ("dw", CH), ("fc2", C)]:
        cci = nc.dram_tensor(f"cci_{name}", [co, 2], F32)
        cco = nc.dram_tensor(f"cco_{name}", [co, 2], F32, addr_space="Shared")
        cc[name] = (cci, cco)

    xmid_sp = nc.dram_tensor("xmid_spill", [C, R], F32)
    zx2_sp = nc.dram_tensor("zx2_spill", [CH, R], F16)

    with tile.TileContext(nc, pool_alloc_mode="queue") as tc:
        _body(nc, tc, xT_in, w_in, pv_in, ident_in, mask_in, convd_in, out_d,
              tok_d, cc, xmid_sp, zx2_sp, dbg, stop_after)
    _fix_multiwaits(nc)
    return nc


def _body(nc, tc, xT_in, w_in, pv_in, ident_in, mask_in, convd_in, out_d,
          tok_d, cc, xmid_sp, zx2_sp, dbg, stop_after=None):
    from contextlib import ExitStack

    # spike DRAM buffers (cross-phase hand-off)
    s_d = {name: nc.dram_tensor(f"s{name}_d", [C, R], F16)
           for name in ("q", "k", "v", "y")}

    # ---------- long-lived small pools ----------
    ctxL = ExitStack()
    const_p = ctxL.enter_context(tc.tile_pool(name="const", bufs=1))
    stat_p = ctxL.enter_context(tc.tile_pool(name="stats", bufs=1))
    scr_p = ctxL.enter_context(tc.tile_pool(name="scr", bufs=2))
    lif_p = ctxL.enter_context(tc.tile_pool(name="lifp", bufs=1))
    ps_mm = ctxL.enter_context(tc.tile_pool(name="psmm", bufs=4, space="PSUM"))
    ps_at = ctxL.enter_context(tc.tile_pool(name="psat", bufs=1, space="PSUM"))

    ident = const_p.tile([128, 128], F16, tag="ident", name="ident")
    nc.sync.dma_start(ident[:], ident_in[:])
    mask = const_p.tile([128, 512], F16, tag="mask", name="mask")
    nc.sync.dma_start(mask[:], mask_in[:])
    pvec = {}
    for name, npt in [("q", 3), ("k", 3), ("v", 3), ("p", 3),
                      ("fc1", 12), ("dw", 6), ("fc2", 3)]:
        pv = const_p.tile([128, 2 * npt], F32, tag=f"pv_{name}",
                          name=f"pv_{name}")
        for i in range(npt):
            nc.sync.dma_start(pv[:, 2 * i:2 * i + 2],
                              pv_in[name][128 * i:128 * (i + 1), :])
        pvec[name] = pv

    # stats: sump, sqp [128, ngrp*npt]; stfin [128, 2*npt] (S,Q); AC [128,2*npt]
    NGRP = {"q": 8, "k": 8, "v": 8, "p": 8, "fc1": 8, "fc2": 8, "dw": 1}
    STT = {}
    for name, npt in [("q", 3), ("k", 3), ("v", 3), ("p", 3),
                      ("fc1", 12), ("dw", 6), ("fc2", 3)]:
        g = NGRP[name]
        STT[name] = (
            stat_p.tile([128, g * npt], F32, tag=f"sum_{name}", name=f"sum_{name}"),
            stat_p.tile([128, g * npt], F32, tag=f"sq_{name}", name=f"sq_{name}"),
            stat_p.tile([128, 2 * npt], F32, tag=f"st_{name}", name=f"st_{name}"),
            stat_p.tile([128, 2 * npt], F32, tag=f"ac_{name}", name=f"ac_{name}"),
        )

    # ---------- helpers ----------
    def emit_linear(name, wt, rhs, co_lo, co_hi, n_ci, z_alloc, z_done):
        """z tiles are f16. PSUM evac on ACT (with sum accum); sq-sums on DVE
        via tensor_tensor_reduce."""
        sump, sqp = STT[name][0], STT[name][1]
        for co in range(co_lo, co_hi):
            z = z_alloc(co)
            for cg in range(2):
                pss = [ps_mm.tile([128, 512], F32, tag="ps", name=f"ps{name}{co}{cg}{j}")
                       for j in range(4)]
                for ci in range(n_ci):
                    for ch in range(4):
                        nc.tensor.matmul(
                            pss[ch][:],
                            lhsT=wt[ci][:, co * 128:(co + 1) * 128],
                            rhs=rhs[ci][:, (cg * 4 + ch) * 512:(cg * 4 + ch + 1) * 512],
                            start=(ci == 0), stop=(ci == n_ci - 1))
                for ch in range(4):
                    g = cg * 4 + ch
                    sl = slice(g * 512, (g + 1) * 512)
                    nc.scalar.activation(z[:, sl], pss[ch][:], ACTF.Copy,
                                         accum_out=sump[:, co * 8 + g:co * 8 + g + 1])
                    sq = scr_p.tile([128, 512], F16, tag="sqscr", name="sqscr")
                    if g % 2 == 0:
                        nc.vector.scalar_tensor_tensor(
                            sq[:], z[:, sl], 1.0, z[:, sl], ALU.mult, ALU.mult,
                            accum_out=sqp[:, co * 8 + g:co * 8 + g + 1])
                    else:
                        nc.scalar.activation(
                            sq[:], pss[ch][:], ACTF.Square,
                            accum_out=sqp[:, co * 8 + g:co * 8 + g + 1])
            z_done(co, z)

    def emit_ar(name, lo, hi):
        """AllReduce stats for ptiles [lo, hi) of `name`."""
        ngrp = NGRP[name]
        sump, sqp, stfin, _ = STT[name]
        for co in range(lo, hi):
            if ngrp == 1:
                nc.vector.tensor_copy(stfin[:, 2 * co:2 * co + 1],
                                      sump[:, co:co + 1])
                nc.vector.tensor_copy(stfin[:, 2 * co + 1:2 * co + 2],
                                      sqp[:, co:co + 1])
            else:
                nc.vector.tensor_reduce(stfin[:, 2 * co:2 * co + 1],
                                        sump[:, co * ngrp:(co + 1) * ngrp],
                                        axis=mybir.AxisListType.X, op=ALU.add)
                nc.vector.tensor_reduce(stfin[:, 2 * co + 1:2 * co + 2],
                                        sqp[:, co * ngrp:(co + 1) * ngrp],
                                        axis=mybir.AxisListType.X, op=ALU.add)
        cci, cco = cc[name]
        dmas = []
        for co in range(lo, hi):
            d = nc.sync.dma_start(cci[128 * co:128 * (co + 1), :],
                                  stfin[:, 2 * co:2 * co + 2])
            dmas.append(d)
        ar = nc.gpsimd.collective_compute(
            "AllReduce", ALU.add, replica_groups=[list(range(NCORES))],
            ins=[cci[128 * lo:128 * hi, :]], outs=[cco[128 * lo:128 * hi, :]])
        for d in dmas:
            add_dep_helper(ar.ins, d.ins, reason="ar waits dma_in")
        for co in range(lo, hi):
            d = nc.sync.dma_start(stfin[:, 2 * co:2 * co + 2],
                                  cco[128 * co:128 * (co + 1), :])
            add_dep_helper(d.ins, ar.ins, reason="readback waits ar")

    def emit_params(name, lo, hi):
        """Per ptile compute A = 0.5*a, C = 0.5*c into AC. All ops on [128,1]
        tiles with tensor_tensor / immediate tensor_scalar (fast paths)."""
        _, _, stfin, AC = STT[name]
        pv = pvec[name]
        for i in range(lo, hi):
            S_ = stfin[:, 2 * i:2 * i + 1]
            Q_ = stfin[:, 2 * i + 1:2 * i + 2]
            g_ = pv[:, 2 * i:2 * i + 1]
            be_ = pv[:, 2 * i + 1:2 * i + 2]
            A_ = AC[:, 2 * i:2 * i + 1]
            C_ = AC[:, 2 * i + 1:2 * i + 2]
            w = scr_p.tile([128, 6], F32, tag="pscr", name="pscr")
            mean, qm, var, sd, inv, a_ = (w[:, j:j + 1] for j in range(6))
            nc.vector.tensor_scalar(mean, S_, 1.0 / COUNT, None, ALU.mult)
            nc.vector.tensor_scalar(qm, Q_, 1.0 / COUNT, None, ALU.mult)
            nc.vector.tensor_tensor(var, mean, mean, ALU.mult)
            nc.vector.tensor_tensor(var, qm, var, ALU.subtract)
            nc.vector.tensor_scalar(var, var, EPS, None, ALU.add)
            nc.scalar.sqrt(sd, var)
            nc.vector.reciprocal(inv, sd)
            nc.vector.tensor_tensor(a_, g_, inv, ALU.mult)
            nc.vector.tensor_scalar(A_, a_, 0.5, None, ALU.mult)
            # C = 0.5*(be - mean*a)
            nc.vector.tensor_tensor(C_, mean, a_, ALU.mult)
            nc.vector.tensor_tensor(C_, be_, C_, ALU.subtract)
            nc.vector.tensor_scalar(C_, C_, 0.5, None, ALU.mult)

    def emit_norm(name, z, yh, pt_off):
        """yh = 0.5*a*z + 0.5*c on ACT (per-partition affine), f16 out."""
        AC = STT[name][3]
        nc.scalar.activation(yh[:], z[:], ACTF.Identity,
                             scale=AC[:, 2 * pt_off:2 * pt_off + 1],
                             bias=AC[:, 2 * pt_off + 1:2 * pt_off + 2])

    def emit_lif(yh, writer, thr=1.0, sdt=F16):
        """LIF over yh [128, R] (already includes the 0.5 input scale).
        u_t = 0.5*u_{t-1}*[u_{t-1} < thr] + yh_t ; writer(t, u) emits spikes.
        All immediate-scalar TS/TT ops."""
        tg = "32" if sdt == F32 else "16"
        u = lif_p.tile([128, TC], sdt, tag="lifu" + tg, name="lifu", bufs=2)
        sc = lif_p.tile([128, TC], sdt, tag="lifsc" + tg, name="lifsc", bufs=2)
        h = lif_p.tile([128, TC], sdt, tag="lifh" + tg, name="lifh", bufs=2)
        for t in range(T):
            ucur = yh[:, 0:TC] if t == 0 else u[:]
            writer(t, ucur)
            if t < T - 1:
                nc.vector.tensor_scalar(sc[:], ucur, thr, 0.5,
                                        ALU.is_lt, ALU.mult)
                nc.vector.tensor_tensor(h[:], ucur, sc[:], ALU.mult)
                nc.vector.tensor_tensor(u[:], h[:],
                                        yh[:, (t + 1) * TC:(t + 2) * TC],
                                        ALU.add)

    def spike_writer(st, thr=1.0, eng=None):
        e = eng if eng is not None else nc.vector
        def w(t, ucur, st=st, thr=thr, e=e):
            e.tensor_scalar(st[:, t * TC:(t + 1) * TC], ucur, thr, None,
                            ALU.is_ge)
        return w

    def dump_rows(nm, row0, t_):
        if nm in dbg:
            nc.sync.dma_start(dbg[nm][row0:row0 + 128, :], t_[:])

    # ============ PHASE 1: q,k,v matmul + AR + LIF -> spikes to DRAM ======
    ctxA = ExitStack()
    pA = ctxA.enter_context(tc.tile_pool(name="pA", bufs=1))
    xT = []
    for i in range(3):
        x = pA.tile([128, R], F32R, tag=f"xT{i}", name=f"xT{i}")
        nc.sync.dma_start(x[:], xT_in[128 * i:128 * (i + 1), :])
        xT.append(x)
    wts = {}
    for name in ("q", "k", "v"):
        wt = []
        for i in range(3):
            w = pA.tile([128, C], F32R, tag=f"w_{name}{i}", name=f"w_{name}{i}")
            nc.sync.dma_start(w[:], w_in[name][128 * i:128 * (i + 1), :])
            wt.append(w)
        wts[name] = wt

    zs = {}
    for name in ("q", "k", "v"):
        zt = []

        def zalloc(co, name=name, zt=zt):
            z = pA.tile([128, R], F16, tag=f"z{co}", name=f"z{name}{co}", bufs=2)
            zt.append(z)
            return z

        emit_linear(name, wts[name], xT, 0, 3, 3, zalloc, lambda co, z: None)
        zs[name] = zt
        emit_ar(name, 0, 3)
        emit_params(name, 0, 3)
    for pt in range(3):
        dump_rows("z_q", 128 * pt, zs["q"][pt])

    for name in ("q", "k", "v"):
        for pt in range(3):
            yh = pA.tile([128, R], F32, tag=f"yh{pt % 2}", name=f"yh{name}{pt}")
            emit_norm(name, zs[name][pt], yh, pt)
            st = pA.tile([128, R], F16, tag=f"spt{pt % 2}", name=f"s{name}{pt}")
            emit_lif(yh, spike_writer(st), sdt=F32)
            nc.sync.dma_start(s_d[name][128 * pt:128 * (pt + 1), :], st[:])
            dump_rows(f"s_{name}", 128 * pt, st)
    ctxA.close()
    if stop_after == 'qkv':
        ctxL.close(); return

    # ============ PHASE 2: transposes + attention + y-LIF ============
    ctxB = ExitStack()
    pB = ctxB.enter_context(tc.tile_pool(name="pB", bufs=1))
    # rm layout: per (pt, tb) a (128, 128) block at col (pt*64+tb)*128;
    # rows 0..63 = transposed spikes (n-major), rows 64..127 stay ZERO so
    # mm1 can contract over the full K=128 (K=64 matmuls hang on this HW).
    rm = {}
    for name in ("k", "v"):
        rmt = pB.tile([128, 6 * R], F16, tag=f"rm_{name}", name=f"rm_{name}")
        nc.gpsimd.memset(rmt[64:128, :], 0.0)
        for pt in range(3):
            srt = pB.tile([128, R], F16, tag=f"skvr{pt % 2}",
                          name=f"r{name}{pt}")
            nc.sync.dma_start(srt[:], s_d[name][128 * pt:128 * (pt + 1), :])
            for grp in range(16):
                pst = ps_at.tile([128, 512], F16, tag="pstr", name="pstr")
                for j in range(4):
                    tb = grp * 4 + j
                    nc.tensor.transpose(pst[0:64, 128 * j:128 * (j + 1)],
                                        srt[:, 64 * tb:64 * (tb + 1)],
                                        ident[:])
                nc.scalar.copy(
                    rmt[0:64, (pt * 64 + grp * 4) * 128:(pt * 64 + grp * 4 + 4) * 128],
                    pst[0:64, :])
        rm[name] = rmt

    sy = []
    for pt in range(3):
        sqr = pB.tile([128, R], F16, tag=f"sqr{pt % 2}", name=f"sqr{pt}")
        nc.sync.dma_start(sqr[:], s_d["q"][128 * pt:128 * (pt + 1), :])
        zy = pB.tile([128, R], F16, tag=f"zy{pt % 2}", name=f"zy{pt}")
        for g4 in range(16):
            mm1ps = ps_at.tile([128, 512], F32, tag="mm1", name="mm1")
            for j in range(4):
                tb = g4 * 4 + j
                base = (pt * 64 + tb) * 128
                nc.tensor.matmul(mm1ps[:, 128 * j:128 * (j + 1)],
                                 lhsT=rm["k"][:, base:base + 128],
                                 rhs=rm["v"][:, base:base + 128],
                                 start=True, stop=True)
            m4 = scr_p.tile([128, 512], F16, tag="m4", name="m4")
            nc.vector.tensor_tensor(m4[:], mm1ps[:], mask[:], ALU.mult)
            yps = ps_at.tile([128, 256], F32, tag="yps", name="yps", bufs=2)
            for j in range(4):
                tb = g4 * 4 + j
                nc.tensor.matmul(yps[:, 64 * j:64 * (j + 1)],
                                 lhsT=m4[:, 128 * j:128 * (j + 1)],
                                 rhs=sqr[:, 64 * tb:64 * (tb + 1)],
                                 start=True, stop=True)
            # evacuate with 0.5 scale: zy holds Y = 0.5 * z_y
            nc.scalar.activation(zy[:, 256 * g4:256 * (g4 + 1)], yps[:],
                                 ACTF.Copy, scale=0.5)
        dump_rows("z_y", 128 * pt, zy)
        syt = pB.tile([128, R], F16, tag=f"sy{pt % 2}", name=f"sy{pt}")
        emit_lif(zy, spike_writer(syt, thr=0.5), thr=0.5)
        nc.sync.dma_start(s_d["y"][128 * pt:128 * (pt + 1), :], syt[:])
        dump_rows("s_y", 128 * pt, syt)
    ctxB.close()
    if stop_after == 'attn':
        ctxL.close(); return

    # ============ PHASE 3: p-linear + xmid ============
    ctxC = ExitStack()
    pC = ctxC.enter_context(tc.tile_pool(name="pC", bufs=1))
    syr = []
    for i in range(3):
        s = pC.tile([128, R], F16, tag=f"syr{i}", name=f"syr{i}")
        nc.sync.dma_start(s[:], s_d["y"][128 * i:128 * (i + 1), :])
        syr.append(s)
    wt_p = []
    for i in range(3):
        w = pC.tile([128, C], F16, tag=f"w_p{i}", name=f"w_p{i}")
        nc.sync.dma_start(w[:], w_in["p"][128 * i:128 * (i + 1), :])
        wt_p.append(w)
    zp = []

    def zalloc_p(co):
        z = pC.tile([128, R], F16, tag=f"zp{co}", name=f"zp{co}")
        zp.append(z)
        return z

    emit_linear("p", wt_p, syr, 0, 3, 3, zalloc_p, lambda co, z: None)
    emit_ar("p", 0, 3)
    emit_params("p", 0, 3)
    for pt in range(3):
        dump_rows("z_p", 128 * pt, zp[pt])
        yh = pC.tile([128, R], F32, tag="yhp", name=f"yhp{pt}", bufs=2)
        emit_norm("p", zp[pt], yh, pt)
        spt = pC.tile([128, R], F16, tag="sptr", name=f"sp{pt}", bufs=2)
        emit_lif(yh, spike_writer(spt), sdt=F32)
        # xr = x + p_spikes ; spill to DRAM (read back in phases 4 and 6)
        xr = pC.tile([128, R], F32, tag=f"xm{pt % 2}", name=f"xm{pt}", bufs=2)
        nc.sync.dma_start(xr[:].bitcast(F32R), xT_in[128 * pt:128 * (pt + 1), :])
        nc.vector.tensor_tensor(xr[:], xr[:], spt[:], ALU.add)
        nc.sync.dma_start(xmid_sp[128 * pt:128 * (pt + 1), :], xr[:])
        dump_rows("xmid", 128 * pt, xr)
    ctxC.close()
    if stop_after == 'p':
        ctxL.close(); return

    # ============ PHASE 4: fc1 (z tiles stay in SBUF) ============
    ctxZ = ExitStack()
    pZ = ctxZ.enter_context(tc.tile_pool(name="pZ", bufs=1))
    ctxD = ExitStack()
    pD = ctxD.enter_context(tc.tile_pool(name="pD", bufs=1))
    wt_fc1 = []
    for i in range(3):
        w = pD.tile([128, HID], F32R, tag=f"wfc1_{i}", name=f"wfc1_{i}")
        nc.sync.dma_start(w[:], w_in["fc1"][128 * i:128 * (i + 1), :])
        wt_fc1.append(w)
    xmid_v = []
    for i in range(3):
        x = pD.tile([128, R], F32R, tag=f"xmid{i}", name=f"xmid{i}")
        nc.sync.dma_start(x[:].bitcast(F32), xmid_sp[128 * i:128 * (i + 1), :])
        xmid_v.append(x)

    zf1 = {}

    def zalloc_f(co):
        if co < 6:
            z = pZ.tile([128, R], F16, tag=f"zf1_{co}", name=f"zf1_{co}")
            zf1[co] = z
        else:
            z = pD.tile([128, R], F16, tag=f"zx2_{co % 2}", name=f"zf1_{co}",
                        bufs=2)
        return z

    def zdone_f(co, z):
        if co >= 6:
            nc.sync.dma_start(zx2_sp[128 * (co - 6):128 * (co - 5), :], z[:])
        if "z_fc1" in dbg:
            nc.sync.dma_start(dbg["z_fc1"][128 * co:128 * (co + 1), :], z[:])

    emit_linear("fc1", wt_fc1, xmid_v, 0, 12, 3, zalloc_f, zdone_f)
    emit_ar("fc1", 0, 12)
    emit_params("fc1", 0, 12)
    ctxD.close()
    if stop_after == 'fc1':
        ctxZ.close(); ctxL.close(); return

    # ============ PHASE 5a: x1-LIF -> spike planes -> PE conv ============
    ctxE = ExitStack()
    pE = ctxE.enter_context(tc.tile_pool(name="pE", bufs=1))
    convd = pE.tile([128, 54 * 128], F16, tag="convd", name="convd")
    for i in range(54):
        nc.sync.dma_start(convd[:, 128 * i:128 * (i + 1)],
                          convd_in[128 * i:128 * (i + 1), :])
    # tap shift offsets in plane space, kh-major to match host convd order
    SHIFTS = [dh * PADW + dw for dh in (-1, 0, 1) for dw in (-1, 0, 1)]

    z_conv = []
    sump_c, sqp_c, _, _ = STT["dw"]
    for i in range(6):
        yh = pE.tile([128, R], F32, tag="yhc", name=f"yhx1_{i}")
        emit_norm("fc1", zf1[i], yh, i)
        xa = pE.tile([128, PADL], F16, tag=f"cxa{i % 2}", name=f"cxa{i}")
        nc.gpsimd.memset(xa[:], 0.0)

        def x1_writer(t, ucur, xa=xa):
            # one strided is_ge into the padded plane per timestep
            xa4 = xa[:, GUARD + t * BS * PADP:GUARD + (t + 1) * BS * PADP] \
                .rearrange("p (f r w) -> p f r w", r=10, w=PADW)[:, :, 1:9, 2:10]
            u4 = ucur.rearrange("p (f h w) -> p f h w", h=8, w=8)
            nc.vector.tensor_scalar(xa4, u4, 1.0, None, ALU.is_ge)

        emit_lif(yh, x1_writer, sdt=F32)

        # 9-tap depthwise conv via diagonal-weight matmuls, plane space
        zpl = pE.tile([128, PLANE], F16, tag="zpl", name=f"zpl{i}")
        for c in range(15):
            cps = ps_mm.tile([128, 512], F32, tag="ps", name=f"cps{i}{c}")
            for k in range(9):
                base = GUARD + c * 512 + SHIFTS[k]
                nc.tensor.matmul(cps[:],
                                 lhsT=convd[:, (i * 9 + k) * 128:(i * 9 + k + 1) * 128],
                                 rhs=xa[:, base:base + 512],
                                 start=(k == 0), stop=(k == 8))
            nc.scalar.copy(zpl[:, 512 * c:512 * (c + 1)], cps[:])
        # repack valid positions (f,1..8,2..9) -> contiguous z_conv + stats
        zc = pZ.tile([128, R], F16, tag=f"zf1_{i}", name=f"zconv{i}")
        zpl4 = zpl[:].rearrange("p (f r w) -> p f r w", r=10, w=PADW)[:, :, 1:9, 2:10]
        nc.scalar.activation(zc[:].rearrange("p (f h w) -> p f h w", h=8, w=8),
                             zpl4, ACTF.Copy,
                             accum_out=sump_c[:, i:i + 1])
        sq = pE.tile([128, R], F16, tag="sqcv", name=f"sqc{i}")
        nc.vector.scalar_tensor_tensor(
            sq[:], zc[:], 1.0, zc[:], ALU.mult, ALU.mult,
            accum_out=sqp_c[:, i:i + 1])
        z_conv.append(zc)
    emit_ar("dw", 0, 6)
    emit_params("dw", 0, 6)
    for i in range(6):
        dump_rows("z_conv", 128 * i, z_conv[i])

    # ============ PHASE 5b: x2-LIF + conv-LIF + gating ============
    gated = []
    for i in range(6):
        zx2 = pE.tile([128, R], F16, tag="zx2r", name=f"zx2r{i}")
        nc.sync.dma_start(zx2[:], zx2_sp[128 * i:128 * (i + 1), :])
        yh2 = pE.tile([128, R], F32, tag="yhc", name=f"yhx2_{i}")
        emit_norm("fc1", zx2, yh2, 6 + i)
        sx2 = pE.tile([128, R], F16, tag="gt", name=f"sx2_{i}")
        emit_lif(yh2, spike_writer(sx2), sdt=F32)
        yhc = pE.tile([128, R], F16, tag="yhcv", name=f"yhcv{i}")
        emit_norm("dw", z_conv[i], yhc, i)
        scv = pE.tile([128, R], F16, tag="scv", name=f"scv{i}")
        emit_lif(yhc, spike_writer(scv))
        dump_rows("s_conv", 128 * i, scv)
        g = pZ.tile([128, R], F16, tag=f"zf1_{i}", name=f"gated{i}")
        nc.vector.tensor_tensor(g[:], scv[:], sx2[:], ALU.mult)
        gated.append(g)
        dump_rows("gated", 128 * i, g)
    ctxE.close()
    if stop_after == 'conv':
        ctxZ.close(); ctxL.close(); return

    # ============ PHASE 6: fc2 + final residual ============
    ctxG = ExitStack()
    pG = ctxG.enter_context(tc.tile_pool(name="pG", bufs=1))
    wt_fc2 = []
    for i in range(6):
        w = pG.tile([128, C], F16, tag=f"wfc2_{i}", name=f"wfc2_{i}")
        nc.sync.dma_start(w[:], w_in["fc2"][128 * i:128 * (i + 1), :])
        wt_fc2.append(w)
    zf2 = []

    def zalloc_g(co):
        z = pG.tile([128, R], F16, tag=f"zf2{co}", name=f"zf2{co}")
        zf2.append(z)
        return z

    emit_linear("fc2", wt_fc2, gated, 0, 3, 6, zalloc_g, lambda co, z: None)
    emit_ar("fc2", 0, 3)
    emit_params("fc2", 0, 3)
    for pt in range(3):
        dump_rows("z_fc2", 128 * pt, zf2[pt])
        yh = pG.tile([128, R], F32, tag="yhg", name=f"yhg{pt}", bufs=2)
        emit_norm("fc2", zf2[pt], yh, pt)
        so = pG.tile([128, R], F16, tag="so", name=f"so{pt}", bufs=2)
        emit_lif(yh, spike_writer(so), sdt=F32)
        xm = pG.tile([128, R], F32, tag=f"xmr{pt % 2}", name=f"xmr{pt}")
        nc.sync.dma_start(xm[:], xmid_sp[128 * pt:128 * (pt + 1), :])
        nc.vector.tensor_tensor(xm[:], xm[:], so[:], ALU.add)
        nc.sync.dma_start(out_d[128 * pt:128 * (pt + 1), :], xm[:])
    if tok_d is not None:
        tk = pG.tile([128, 1], F32, tag="tok", name="tk")
        nc.vector.memset(tk[:], 1.0)
        nc.sync.dma_start(tok_d[:], tk[:])
    ctxG.close()
    ctxZ.close()
    ctxL.close()


# ---------------- host glue ----------------

def _prep_inputs(inputs):
    x = np.asarray(inputs['x'], np.float32)
    xr = x.reshape(T, B, N, C)
    ident = np.eye(128, dtype=np.float16)
    mask = np.zeros((128, 512), np.float16)
    for blk in range(4):
        for h in range(4):
            mask[h * 32:(h + 1) * 32,
                 blk * 128 + h * 32:blk * 128 + (h + 1) * 32] = 0.125
    common = {"ident": ident, "mask": mask}
    for name in ("q", "k", "v", "p", "fc1", "fc2"):
        wdt = np.float16 if name in ("p", "fc2") else np.float32
        common[f"w_{name}"] = np.ascontiguousarray(
            np.asarray(inputs[name + "_w"]).T).astype(wdt)
    for name in ("q", "k", "v", "p", "fc1", "fc2"):
        common[f"pv_{name}"] = np.ascontiguousarray(np.stack(
            [np.asarray(inputs[name + "_g"], np.float32),
             np.asarray(inputs[name + "_be"], np.float32)], 1))
    common["pv_dw"] = np.ascontiguousarray(np.stack(
        [np.asarray(inputs["dw_g"], np.float32),
         np.asarray(inputs["dw_be"], np.float32)], 1))
    kv = np.asarray(inputs["dw_k"], np.float32).reshape(CH, 9)
    convd = np.zeros((54 * 128, 128), np.float16)
    for i in range(6):
        for k in range(9):
            blk = np.diag(kv[i * 128:(i + 1) * 128, k]).astype(np.float16)
            convd[(i * 9 + k) * 128:(i * 9 + k + 1) * 128, :] = blk
    common["convd"] = convd

    maps = []
    for c in range(NCORES):
        shard = xr[:, c * BS:(c + 1) * BS]
        xt = np.ascontiguousarray(shard.reshape(R, C).T)
        m = dict(common)
        m["xT"] = xt
        maps.append(m)
    return maps


_CACHE = {}


def _get_runner(debug_taps=False, timing=False, stop_after=None):
    key = (debug_taps, timing, stop_after)
    if key not in _CACHE:
        from runner_embed import SpmdRunner
        nc = build_kernel(debug_taps, timing, stop_after)
        _CACHE[key] = SpmdRunner(nc, NCORES)
    return _CACHE[key]


def kernel(**inputs):
    r = _get_runner()
    maps = _prep_inputs(inputs)
    args = r.prep(maps)
    outs = r.run(args)
    res = r.results(outs)
    full = np.empty((T, B, N, C), np.float32)
    for c in range(NCORES):
        o = res[c]["out"]
        full[:, c * BS:(c + 1) * BS] = o.T.reshape(T, BS, N, C)
    return np.ascontiguousarray(full.reshape(T * B, N, C))


# ---- embedded SPMD runner module ----
import types
runner_embed = types.ModuleType("runner_embed")
sys.modules["runner_embed"] = runner_embed
exec(r'''
import sys
sys.path.insert(0, '/opt/trn_rl_repo')
import numpy as np
import jax
from jax.sharding import Mesh, PartitionSpec, NamedSharding
from jax.experimental.shard_map import shard_map
import concourse.bass as bass
import concourse.mybir as mybir
from concourse.bass2jax import _bass_exec_p, install_neuronx_cc_hook, partition_id_tensor


class SpmdRunner:
    def __init__(self, nc, n_cores):
        install_neuronx_cc_hook()
        self.nc = nc
        self.n_cores = n_cores
        partition_name = nc.partition_id_tensor.name if nc.partition_id_tensor else None
        in_names, out_names, out_avals, zero_outs = [], [], [], []
        for alloc in nc.m.functions[0].allocations:
            if not isinstance(alloc, mybir.MemoryLocationSet):
                continue
            name = alloc.memorylocations[0].name
            if alloc.kind == "ExternalInput":
                if name != partition_name:
                    in_names.append(name)
            elif alloc.kind == "ExternalOutput":
                shape = tuple(alloc.tensor_shape)
                dtype = mybir.dt.np(alloc.dtype)
                out_names.append(name)
                out_avals.append(jax.core.ShapedArray(shape, dtype))
                zero_outs.append(np.zeros(shape, dtype))
        self.in_names, self.out_names = in_names, out_names
        self.out_avals, self.zero_outs = out_avals, zero_outs
        n_params = len(in_names)
        n_outs = len(out_avals)
        all_in_names = list(in_names) + list(out_names)
        if partition_name is not None:
            all_in_names.append(partition_name)
        self.n_params = n_params

        def _body(*args):
            operands = list(args)
            if partition_name is not None:
                operands.append(partition_id_tensor())
            outs = _bass_exec_p.bind(
                *operands, out_avals=tuple(out_avals),
                in_names=tuple(all_in_names), out_names=tuple(out_names),
                lowering_input_output_aliases=(),
                sim_require_finite=True, sim_require_nnan=True, nc=nc)
            return tuple(outs)

        devices = jax.devices()[:n_cores]
        assert len(devices) == n_cores
        mesh = Mesh(np.asarray(devices), ("core",))
        self.mesh = mesh
        in_specs = (PartitionSpec("core"),) * (n_params + n_outs)
        out_specs = (PartitionSpec("core"),) * n_outs
        self.fn = jax.jit(
            shard_map(_body, mesh=mesh, in_specs=in_specs,
                      out_specs=out_specs, check_rep=False),
            keep_unused=True)

    def prep(self, in_maps):
        per_core = [[np.asarray(m[name]) for name in self.in_names]
                    for m in in_maps]
        concat_in = [np.concatenate([per_core[c][i] for c in range(self.n_cores)], axis=0)
                     for i in range(self.n_params)]
        concat_zeros = [np.zeros((self.n_cores * z.shape[0], *z.shape[1:]), z.dtype)
                        for z in self.zero_outs]
        sh = NamedSharding(self.mesh, PartitionSpec("core"))
        return [jax.device_put(a, sh) for a in concat_in + concat_zeros]

    def run(self, args):
        outs = self.fn(*args)
        jax.block_until_ready(outs)
        return outs

    def results(self, outs):
        res = []
        for c in range(self.n_cores):
            res.append({name: np.asarray(outs[i]).reshape(self.n_cores, *self.out_avals[i].shape)[c]
                        for i, name in enumerate(self.out_names)})
        return res

    def time_it(self, args, iters=20, warmup=3):
        import time
        for _ in range(warmup):
            self.run(args)
        ts = []
        for _ in range(iters):
            t0 = time.perf_counter()
            self.run(args)
            ts.append(time.perf_counter() - t0)
        ts = np.array(ts)
        return dict(min=ts.min(), median=float(np.median(ts)), mean=ts.mean())
''', runner_embed.__dict__)



# revision 13
# speedup vs baseline: 13.4734x; 1.3734x over previous
"""Trainium2 Bass kernel for nn_Block_76519137345684 (Spikformer-style block:
spiking self-attention + spiking gated MLP with training-mode BatchNorm).

Strategy
- Data-parallel over batch B across 8 NeuronCores (16 batch each). BN batch
  statistics (per-channel sum / sum-of-squares) are AllReduced across cores.
- Activations live channel-on-partition: (C, rows) with rows
  r = ((t*16 + b)*64 + n); LIF timesteps are contiguous 1024-column slices.
- BN application is FUSED into the LIF recurrence on DVE in f16:
      yh_t = A*z_t + C   (per-partition A=0.5*a, C=0.5*c as tensor_scalar
                          AP operands - scalar APs don't break DVE fast modes)
      u_t  = 0.5*u_{t-1}*[u_{t-1} < thr] + yh_t,  s_t = [u_t >= thr]
- Attention uses associativity: y = q @ (k^T v) * scale; per-head block
  structure enforced with a 0.125-scaled block-diagonal mask.
- Depthwise 3x3 conv on the TENSOR engine: spikes written into a zero-padded
  plane (10 rows x 10 cols per frame); 9 taps = 9 PSUM-accumulated matmuls
  with diagonal per-channel weights against shifted plane views. PSUM chunks
  are 4 frames (400 cols) and evacuate DIRECTLY to the contiguous z layout
  via strided ACT copy with fused stats accumulation.
- Matmul dtypes: f32r for continuous-input layers (q,k,v,fc1), fp16 for
  binary-input layers (p, fc2, conv) and attention.
- xmid (residual) stays in SBUF through fc1; spilled to DRAM only for the
  final residual read.
"""
import sys
sys.path.insert(0, '/opt/trn_rl_repo')
import numpy as np

import concourse.bass as bass
import concourse.mybir as mybir
import concourse.tile as tile
from concourse.tile import add_dep_helper

T, B, N, C = 4, 128, 64, 384
HID, CH, HEADS, HD = 1536, 768, 12, 32
NCORES = 8
BS = B // NCORES
R = T * BS * N              # 4096 rows per core
TC = BS * N                 # 1024 cols per timestep
COUNT = T * B * N           # 32768 rows globally (BN stat count)
EPS = 1e-5
PADW = 10
PADP = PADW * 10            # 100 per frame plane
NFR = T * BS                # 64 frames
GUARD = 16
PLANE = NFR * PADP          # 6400
PADL = GUARD + PLANE + GUARD
FR_CH = 4                   # frames per conv psum chunk
CCH = NFR // FR_CH          # 16 chunks per tile
PCOLS = FR_CH * PADP        # 400 plane cols per chunk
ZCOLS = FR_CH * 64          # 256 z cols per chunk

F32 = mybir.dt.float32
F32R = mybir.dt.float32r
F16 = mybir.dt.float16
ALU = mybir.AluOpType
ACTF = mybir.ActivationFunctionType

_ctr = [0]


def _fix_multiwaits(nc):
    """walrus here accepts max 1 sync-wait per instruction: split extras
    onto same-engine NOPs."""
    for f in nc.m.functions:
        for bb in f.blocks:
            new_insts = []
            for inst in bb.instructions:
                si = inst.sync_info
                ow = list(si.on_wait) if (si and si.on_wait) else []
                if len(ow) > 1:
                    for w in ow[:-1]:
                        _ctr[0] += 1
                        new_insts.append(mybir.InstNoOp(
                            name=f"I-waitnop-{_ctr[0]}", engine=inst.engine,
                            sync_info=mybir.SyncInfo(on_wait=[w], on_update=[]),
                            bass_nofuse=True))
                    si.on_wait = [ow[-1]]
                new_insts.append(inst)
            bb.instructions[:] = new_insts


def build_kernel(debug_taps=False, timing=False, stop_after=None):
    nc = bass.Bass("TRN2", target_bir_lowering=False, debug=False,
                   num_devices=NCORES)

    xT_in = nc.declare_dram_parameter("xT", [C, R], F32R, isOutput=False)
    w_in = {}
    for name, ci, co, dt in [("q", C, C, F32R), ("k", C, C, F32R),
                             ("v", C, C, F32R), ("p", C, C, F16),
                             ("fc1", C, HID, F32R), ("fc2", CH, C, F16)]:
        w_in[name] = nc.declare_dram_parameter(f"w_{name}", [ci, co], dt,
                                               isOutput=False)
    pv_in = {}
    for name, co in [("q", C), ("k", C), ("v", C), ("p", C),
                     ("fc1", HID), ("dw", CH), ("fc2", C)]:
        pv_in[name] = nc.declare_dram_parameter(f"pv_{name}", [co, 2], F32,
                                                isOutput=False)
    ident_in = nc.declare_dram_parameter("ident", [128, 128], F16, isOutput=False)
    mask_in = nc.declare_dram_parameter("mask", [128, 512], F16, isOutput=False)
    # 54 diagonal [128,128] f16 weight matrices: (tile i, tap k) at row
    # (i*9+k)*128
    convd_in = nc.declare_dram_parameter("convd", [54 * 128, 128], F16,
                                         isOutput=False)
    if timing:
        out_d = nc.dram_tensor("out", [C, R], F32)
        tok_d = nc.declare_dram_parameter("tok", [128, 1], F32, isOutput=True)
    else:
        out_d = nc.declare_dram_parameter("out", [C, R], F32, isOutput=True)
        tok_d = None

    dbg = {}
    if debug_taps:
        for nm, npt, dt in [("z_q", 3, F16), ("s_q", 3, F16), ("s_k", 3, F16),
                            ("s_v", 3, F16), ("z_y", 3, F16), ("s_y", 3, F16),
                            ("z_p", 3, F16), ("xmid", 3, F32),
                            ("z_fc1", 12, F16), ("z_conv", 6, F16),
                            ("s_conv", 6, F16), ("gated", 6, F16),
                            ("z_fc2", 3, F16)]:
            dbg[nm] = nc.declare_dram_parameter(f"dbg_{nm}", [npt * 128, R],
                                                dt, isOutput=True)

    cc = {}
    for name, co in [("q", C), ("k", C), ("v", C), ("p", C),
                     ("fc1", HID), ("dw", CH), ("fc2", C)]:
        cci = nc.dram_tensor(f"cci_{name}", [co, 2], F32)
        cco = nc.dram_tensor(f"cco_{name}", [co, 2], F32, addr_space="Shared")
        cc[name] = (cci, cco)

    xmid_sp = nc.dram_tensor("xmid_spill", [C, R], F32)
    zx2_sp = nc.dram_tensor("zx2_spill", [CH, R], F16)

    with tile.TileContext(nc, pool_alloc_mode="queue") as tc:
        _body(nc, tc, xT_in, w_in, pv_in, ident_in, mask_in, convd_in, out_d,
              tok_d, cc, xmid_sp, zx2_sp, dbg, stop_after)
    _fix_multiwaits(nc)
    return nc


def _body(nc, tc, xT_in, w_in, pv_in, ident_in, mask_in, convd_in, out_d,
          tok_d, cc, xmid_sp, zx2_sp, dbg, stop_after=None):
    from contextlib import ExitStack

    # spike DRAM buffers (cross-phase hand-off)
    s_d = {name: nc.dram_tensor(f"s{name}_d", [C, R], F16)
           for name in ("q", "k", "v", "y")}

    # ---------- long-lived small pools ----------
    ctxL = ExitStack()
    const_p = ctxL.enter_context(tc.tile_pool(name="const", bufs=1))
    stat_p = ctxL.enter_context(tc.tile_pool(name="stats", bufs=1))
    scr_p = ctxL.enter_context(tc.tile_pool(name="scr", bufs=2))
    lif_p = ctxL.enter_context(tc.tile_pool(name="lifp", bufs=1))
    ps_mm = ctxL.enter_context(tc.tile_pool(name="psmm", bufs=4, space="PSUM"))
    ps_at = ctxL.enter_context(tc.tile_pool(name="psat", bufs=1, space="PSUM"))

    ident = const_p.tile([128, 128], F16, tag="ident", name="ident")
    nc.sync.dma_start(ident[:], ident_in[:])
    mask = const_p.tile([128, 512], F16, tag="mask", name="mask")
    nc.sync.dma_start(mask[:], mask_in[:])
    pvec = {}
    for name, npt in [("q", 3), ("k", 3), ("v", 3), ("p", 3),
                      ("fc1", 12), ("dw", 6), ("fc2", 3)]:
        pv = const_p.tile([128, 2 * npt], F32, tag=f"pv_{name}",
                          name=f"pv_{name}")
        for i in range(npt):
            nc.sync.dma_start(pv[:, 2 * i:2 * i + 2],
                              pv_in[name][128 * i:128 * (i + 1), :])
        pvec[name] = pv

    # stats: sump, sqp [128, ngrp*npt]; stfin [128, 2*npt] (S,Q); AC [128,2*npt]
    NGRP = {"q": 8, "k": 8, "v": 8, "p": 8, "fc1": 8, "fc2": 8, "dw": 16}
    STT = {}
    for name, npt in [("q", 3), ("k", 3), ("v", 3), ("p", 3),
                      ("fc1", 12), ("dw", 6), ("fc2", 3)]:
        g = NGRP[name]
        STT[name] = (
            stat_p.tile([128, g * npt], F32, tag=f"sum_{name}", name=f"sum_{name}"),
            stat_p.tile([128, g * npt], F32, tag=f"sq_{name}", name=f"sq_{name}"),
            stat_p.tile([128, 2 * npt], F32, tag=f"st_{name}", name=f"st_{name}"),
            stat_p.tile([128, 2 * npt], F32, tag=f"ac_{name}", name=f"ac_{name}"),
        )

    # ---------- helpers ----------
    def emit_linear(name, wt, rhs, co_lo, co_hi, n_ci, z_alloc, z_done):
        """z tiles are f16. PSUM evac on ACT (with sum accum); sq-sums on DVE
        via tensor_tensor_reduce."""
        sump, sqp = STT[name][0], STT[name][1]
        for co in range(co_lo, co_hi):
            z = z_alloc(co)
            for cg in range(2):
                pss = [ps_mm.tile([128, 512], F32, tag="ps", name=f"ps{name}{co}{cg}{j}")
                       for j in range(4)]
                for ci in range(n_ci):
                    for ch in range(4):
                        nc.tensor.matmul(
                            pss[ch][:],
                            lhsT=wt[ci][:, co * 128:(co + 1) * 128],
                            rhs=rhs[ci][:, (cg * 4 + ch) * 512:(cg * 4 + ch + 1) * 512],
                            start=(ci == 0), stop=(ci == n_ci - 1))
                for ch in range(4):
                    g = cg * 4 + ch
                    sl = slice(g * 512, (g + 1) * 512)
                    nc.scalar.activation(z[:, sl], pss[ch][:], ACTF.Copy,
                                         accum_out=sump[:, co * 8 + g:co * 8 + g + 1])
                    sq = scr_p.tile([128, 512], F16, tag="sqscr", name="sqscr")
                    if g % 2 == 0:
                        nc.vector.scalar_tensor_tensor(
                            sq[:], z[:, sl], 1.0, z[:, sl], ALU.mult, ALU.mult,
                            accum_out=sqp[:, co * 8 + g:co * 8 + g + 1])
                    else:
                        nc.scalar.activation(
                            sq[:], pss[ch][:], ACTF.Square,
                            accum_out=sqp[:, co * 8 + g:co * 8 + g + 1])
            z_done(co, z)

    def emit_ar(name, lo, hi):
        """AllReduce stats for ptiles [lo, hi) of `name`."""
        ngrp = NGRP[name]
        sump, sqp, stfin, _ = STT[name]
        for co in range(lo, hi):
            nc.vector.tensor_reduce(stfin[:, 2 * co:2 * co + 1],
                                    sump[:, co * ngrp:(co + 1) * ngrp],
                                    axis=mybir.AxisListType.X, op=ALU.add)
            nc.vector.tensor_reduce(stfin[:, 2 * co + 1:2 * co + 2],
                                    sqp[:, co * ngrp:(co + 1) * ngrp],
                                    axis=mybir.AxisListType.X, op=ALU.add)
        cci, cco = cc[name]
        dmas = []
        for co in range(lo, hi):
            d = nc.sync.dma_start(cci[128 * co:128 * (co + 1), :],
                                  stfin[:, 2 * co:2 * co + 2])
            dmas.append(d)
        ar = nc.gpsimd.collective_compute(
            "AllReduce", ALU.add, replica_groups=[list(range(NCORES))],
            ins=[cci[128 * lo:128 * hi, :]], outs=[cco[128 * lo:128 * hi, :]])
        for d in dmas:
            add_dep_helper(ar.ins, d.ins, reason="ar waits dma_in")
        for co in range(lo, hi):
            d = nc.sync.dma_start(stfin[:, 2 * co:2 * co + 2],
                                  cco[128 * co:128 * (co + 1), :])
            add_dep_helper(d.ins, ar.ins, reason="readback waits ar")

    def emit_params(name, lo, hi):
        """Per ptile compute A = 0.5*a, C = 0.5*c into AC. All ops on [128,1]
        tiles with tensor_tensor / immediate tensor_scalar (fast paths)."""
        _, _, stfin, AC = STT[name]
        pv = pvec[name]
        for i in range(lo, hi):
            S_ = stfin[:, 2 * i:2 * i + 1]
            Q_ = stfin[:, 2 * i + 1:2 * i + 2]
            g_ = pv[:, 2 * i:2 * i + 1]
            be_ = pv[:, 2 * i + 1:2 * i + 2]
            A_ = AC[:, 2 * i:2 * i + 1]
            C_ = AC[:, 2 * i + 1:2 * i + 2]
            w = scr_p.tile([128, 6], F32, tag="pscr", name="pscr")
            mean, qm, var, sd, inv, a_ = (w[:, j:j + 1] for j in range(6))
            nc.vector.tensor_scalar(mean, S_, 1.0 / COUNT, None, ALU.mult)
            nc.vector.tensor_scalar(qm, Q_, 1.0 / COUNT, None, ALU.mult)
            nc.vector.tensor_tensor(var, mean, mean, ALU.mult)
            nc.vector.tensor_tensor(var, qm, var, ALU.subtract)
            nc.vector.tensor_scalar(var, var, EPS, None, ALU.add)
            nc.scalar.sqrt(sd, var)
            nc.vector.reciprocal(inv, sd)
            nc.vector.tensor_tensor(a_, g_, inv, ALU.mult)
            nc.vector.tensor_scalar(A_, a_, 0.5, None, ALU.mult)
            # C = 0.5*(be - mean*a)
            nc.vector.tensor_tensor(C_, mean, a_, ALU.mult)
            nc.vector.tensor_tensor(C_, be_, C_, ALU.subtract)
            nc.vector.tensor_scalar(C_, C_, 0.5, None, ALU.mult)

    def emit_lif(z, writer, name=None, pt=0, thr=1.0):
        """Fused norm+LIF over z [128, R] f16 on DVE.
        If name given: yh_t = A*z_t + C (per-partition AP scalars from AC);
        else z is already yh (includes any input scale).
        u_t = 0.5*u_{t-1}*[u_{t-1} < thr] + yh_t ; writer(t, u_ap) emits
        spikes."""
        AC = STT[name][3] if name is not None else None
        u = lif_p.tile([128, TC], F16, tag="lifu", name="lifu", bufs=2)
        sc = lif_p.tile([128, TC], F16, tag="lifsc", name="lifsc", bufs=2)
        h = lif_p.tile([128, TC], F16, tag="lifh", name="lifh", bufs=2)
        y2 = lif_p.tile([128, TC], F16, tag="lify", name="lify", bufs=2)

        def yh_chunk(dst, t):
            if AC is None:
                return z[:, t * TC:(t + 1) * TC]
            nc.vector.tensor_scalar(dst[:], z[:, t * TC:(t + 1) * TC],
                                    AC[:, 2 * pt:2 * pt + 1],
                                    AC[:, 2 * pt + 1:2 * pt + 2],
                                    ALU.mult, ALU.add)
            return dst[:]

        ucur = yh_chunk(u, 0)
        for t in range(T):
            writer(t, ucur)
            if t < T - 1:
                nc.vector.tensor_scalar(sc[:], ucur, thr, 0.5,
                                        ALU.is_lt, ALU.mult)
                nc.vector.tensor_tensor(h[:], ucur, sc[:], ALU.mult)
                yn = yh_chunk(y2, t + 1)
                nc.vector.tensor_tensor(u[:], h[:], yn, ALU.add)
                ucur = u[:]

    def spike_writer(st, thr=1.0, eng=None):
        e = eng if eng is not None else nc.vector
        def w(t, ucur, st=st, thr=thr, e=e):
            e.tensor_scalar(st[:, t * TC:(t + 1) * TC], ucur, thr, None,
                            ALU.is_ge)
        return w

    def dump_rows(nm, row0, t_):
        if nm in dbg:
            nc.sync.dma_start(dbg[nm][row0:row0 + 128, :], t_[:])

    # ============ PHASE 1: q,k,v matmul + AR + LIF -> spikes to DRAM ======
    ctxA = ExitStack()
    pA = ctxA.enter_context(tc.tile_pool(name="pA", bufs=1))
    xT = []
    for i in range(3):
        x = pA.tile([128, R], F32R, tag=f"xT{i}", name=f"xT{i}")
        nc.sync.dma_start(x[:], xT_in[128 * i:128 * (i + 1), :])
        xT.append(x)
    wts = {}
    for name in ("q", "k", "v"):
        wt = []
        for i in range(3):
            w = pA.tile([128, C], F32R, tag=f"w_{name}{i}", name=f"w_{name}{i}")
            nc.sync.dma_start(w[:], w_in[name][128 * i:128 * (i + 1), :])
            wt.append(w)
        wts[name] = wt

    zs = {}
    for name in ("q", "k", "v"):
        zt = []

        def zalloc(co, name=name, zt=zt):
            z = pA.tile([128, R], F16, tag=f"z{name}{co}", name=f"z{name}{co}",
                        bufs=1)
            zt.append(z)
            return z

        emit_linear(name, wts[name], xT, 0, 3, 3, zalloc, lambda co, z: None)
        zs[name] = zt
        emit_ar(name, 0, 3)
        emit_params(name, 0, 3)
    for pt in range(3):
        dump_rows("z_q", 128 * pt, zs["q"][pt])

    # LIF order k, v first so attention transposes can start while q runs
    for name in ("k", "v", "q"):
        for pt in range(3):
            st = pA.tile([128, R], F16, tag=f"spt{pt % 2}", name=f"s{name}{pt}",
                         bufs=2)
            emit_lif(zs[name][pt], spike_writer(st), name=name, pt=pt)
            nc.sync.dma_start(s_d[name][128 * pt:128 * (pt + 1), :], st[:])
            dump_rows(f"s_{name}", 128 * pt, st)
    ctxA.close()
    if stop_after == 'qkv':
        ctxL.close(); return

    # ============ PHASE 2: transposes + attention + y-LIF ============
    ctxB = ExitStack()
    pB = ctxB.enter_context(tc.tile_pool(name="pB", bufs=1))
    # rm layout: per (pt, tb) a (128, 128) block at col (pt*64+tb)*128;
    # rows 0..63 = transposed spikes (n-major), rows 64..127 stay ZERO so
    # mm1 can contract over the full K=128 (K=64 matmuls hang on this HW).
    rm = {}
    for name in ("k", "v"):
        rmt = pB.tile([128, 6 * R], F16, tag=f"rm_{name}", name=f"rm_{name}")
        nc.vector.memset(rmt[64:128, :], 0.0)
        for pt in range(3):
            srt = pB.tile([128, R], F16, tag=f"skvr{pt % 2}",
                          name=f"r{name}{pt}")
            nc.sync.dma_start(srt[:], s_d[name][128 * pt:128 * (pt + 1), :])
            for grp in range(8):
                pst = ps_at.tile([128, 1024], F16, tag="pstr", name="pstr")
                for j in range(8):
                    tb = grp * 8 + j
                    nc.tensor.transpose(pst[0:64, 128 * j:128 * (j + 1)],
                                        srt[:, 64 * tb:64 * (tb + 1)],
                                        ident[:])
                nc.scalar.copy(
                    rmt[0:64, (pt * 64 + grp * 8) * 128:(pt * 64 + grp * 8 + 8) * 128],
                    pst[0:64, :])
        rm[name] = rmt

    sy = []
    for pt in range(3):
        sqr = pB.tile([128, R], F16, tag=f"sqr{pt % 2}", name=f"sqr{pt}")
        nc.sync.dma_start(sqr[:], s_d["q"][128 * pt:128 * (pt + 1), :])
        zy = pB.tile([128, R], F16, tag=f"zy{pt % 2}", name=f"zy{pt}")
        for g4 in range(16):
            mm1ps = ps_at.tile([128, 512], F32, tag="mm1", name="mm1")
            for j in range(4):
                tb = g4 * 4 + j
                base = (pt * 64 + tb) * 128
                nc.tensor.matmul(mm1ps[:, 128 * j:128 * (j + 1)],
                                 lhsT=rm["k"][:, base:base + 128],
                                 rhs=rm["v"][:, base:base + 128],
                                 start=True, stop=True)
            m4 = scr_p.tile([128, 512], F16, tag="m4", name="m4")
            nc.vector.tensor_tensor(m4[:], mm1ps[:], mask[:], ALU.mult)
            yps = ps_at.tile([128, 256], F32, tag="yps", name="yps", bufs=2)
            for j in range(4):
                tb = g4 * 4 + j
                nc.tensor.matmul(yps[:, 64 * j:64 * (j + 1)],
                                 lhsT=m4[:, 128 * j:128 * (j + 1)],
                                 rhs=sqr[:, 64 * tb:64 * (tb + 1)],
                                 start=True, stop=True)
            # evacuate with 0.5 scale: zy holds Y = 0.5 * z_y
            nc.scalar.activation(zy[:, 256 * g4:256 * (g4 + 1)], yps[:],
                                 ACTF.Copy, scale=0.5)
        dump_rows("z_y", 128 * pt, zy)
        syt = pB.tile([128, R], F16, tag=f"sy{pt % 2}", name=f"sy{pt}")
        emit_lif(zy, spike_writer(syt, thr=0.5), thr=0.5)
        nc.sync.dma_start(s_d["y"][128 * pt:128 * (pt + 1), :], syt[:])
        dump_rows("s_y", 128 * pt, syt)
    ctxB.close()
    if stop_after == 'attn':
        ctxL.close(); return

    # ============ PHASE 3: p-linear + xmid (xr stays in SBUF for fc1) =====
    ctxZ = ExitStack()
    pZ = ctxZ.enter_context(tc.tile_pool(name="pZ", bufs=1))
    ctxC2 = ExitStack()
    pC2 = ctxC2.enter_context(tc.tile_pool(name="pC2", bufs=1))
    ctxC1 = ExitStack()
    pC1 = ctxC1.enter_context(tc.tile_pool(name="pC1", bufs=1))
    syr = []
    for i in range(3):
        s = pC1.tile([128, R], F16, tag=f"syr{i}", name=f"syr{i}")
        nc.sync.dma_start(s[:], s_d["y"][128 * i:128 * (i + 1), :])
        syr.append(s)
    wt_p = []
    for i in range(3):
        w = pC1.tile([128, C], F16, tag=f"w_p{i}", name=f"w_p{i}")
        nc.sync.dma_start(w[:], w_in["p"][128 * i:128 * (i + 1), :])
        wt_p.append(w)
    zp = []

    def zalloc_p(co):
        z = pC1.tile([128, R], F16, tag=f"zp{co}", name=f"zp{co}")
        zp.append(z)
        return z

    emit_linear("p", wt_p, syr, 0, 3, 3, zalloc_p, lambda co, z: None)
    emit_ar("p", 0, 3)
    emit_params("p", 0, 3)
    xr_t = []
    for pt in range(3):
        dump_rows("z_p", 128 * pt, zp[pt])
        spt = pC1.tile([128, R], F16, tag="sptr", name=f"sp{pt}", bufs=2)
        emit_lif(zp[pt], spike_writer(spt), name="p", pt=pt)
        # xr = x + p_spikes ; kept in SBUF for fc1, spilled for final residual
        # (xr written as f32r so the fc1 f32r matmul can consume it directly)
        xr = pC2.tile([128, R], F32R, tag=f"xm{pt}", name=f"xm{pt}")
        nc.sync.dma_start(xr[:], xT_in[128 * pt:128 * (pt + 1), :])
        nc.vector.tensor_tensor(xr[:], xr[:].bitcast(F32), spt[:], ALU.add)
        nc.sync.dma_start(xmid_sp[128 * pt:128 * (pt + 1), :],
                          xr[:].bitcast(F32))
        dump_rows("xmid", 128 * pt, xr[:].bitcast(F32))
        xr_t.append(xr)
    ctxC1.close()
    if stop_after == 'p':
        ctxC2.close(); ctxZ.close(); ctxL.close(); return

    # ============ PHASE 4: fc1 (z tiles stay in SBUF) ============
    ctxD = ExitStack()
    pD = ctxD.enter_context(tc.tile_pool(name="pD", bufs=1))
    wt_fc1 = []
    for i in range(3):
        w = pD.tile([128, HID], F32R, tag=f"wfc1_{i}", name=f"wfc1_{i}")
        nc.sync.dma_start(w[:], w_in["fc1"][128 * i:128 * (i + 1), :])
        wt_fc1.append(w)
    xmid_v = [x[:] for x in xr_t]

    zf1 = {}

    def zalloc_f(co):
        if co < 6:
            z = pZ.tile([128, R], F16, tag=f"zf1_{co}", name=f"zf1_{co}")
            zf1[co] = z
        else:
            z = pD.tile([128, R], F16, tag=f"zx2_{co % 2}", name=f"zf1_{co}",
                        bufs=2)
        return z

    def zdone_f(co, z):
        if co >= 6:
            nc.sync.dma_start(zx2_sp[128 * (co - 6):128 * (co - 5), :], z[:])
        if "z_fc1" in dbg:
            nc.sync.dma_start(dbg["z_fc1"][128 * co:128 * (co + 1), :], z[:])

    emit_linear("fc1", wt_fc1, xmid_v, 0, 12, 3, zalloc_f, zdone_f)
    emit_ar("fc1", 0, 12)
    emit_params("fc1", 0, 12)
    ctxD.close()
    ctxC2.close()
    if stop_after == 'fc1':
        ctxZ.close(); ctxL.close(); return

    # ============ PHASE 5a: x1-LIF -> spike planes -> PE conv ============
    ctxE = ExitStack()
    pE = ctxE.enter_context(tc.tile_pool(name="pE", bufs=1))
    convd = pE.tile([128, 54 * 128], F16, tag="convd", name="convd")
    for i in range(54):
        nc.sync.dma_start(convd[:, 128 * i:128 * (i + 1)],
                          convd_in[128 * i:128 * (i + 1), :])
    # tap shift offsets in plane space, kh-major to match host convd order
    SHIFTS = [dh * PADW + dw for dh in (-1, 0, 1) for dw in (-1, 0, 1)]

    z_conv = []
    sump_c, sqp_c, _, _ = STT["dw"]
    for i in range(6):
        xa = pE.tile([128, PADL], F16, tag=f"cxa{i % 2}", name=f"cxa{i}")
        if i < 2:
            nc.vector.memset(xa[:], 0.0)

        def x1_writer(t, ucur, xa=xa):
            # one strided is_ge into the padded plane per timestep
            xa4 = xa[:, GUARD + t * BS * PADP:GUARD + (t + 1) * BS * PADP] \
                .rearrange("p (f r w) -> p f r w", r=10, w=PADW)[:, :, 1:9, 1:9]
            u4 = ucur.rearrange("p (f h w) -> p f h w", h=8, w=8)
            nc.vector.tensor_scalar(xa4, u4, 1.0, None, ALU.is_ge)

        emit_lif(zf1[i], x1_writer, name="fc1", pt=i)

        # 9-tap depthwise conv via diagonal-weight matmuls; psum chunks of
        # FR_CH frames evacuate directly to contiguous z layout with stats
        zc = pZ.tile([128, R], F16, tag=f"zf1_{i}", name=f"zconv{i}")
        for c in range(CCH):
            cp = ps_mm.tile([128, 512], F32, tag="ps", name=f"cps{i}{c}")
            for k in range(9):
                base = GUARD + c * PCOLS + SHIFTS[k]
                nc.tensor.matmul(cp[:, 0:PCOLS],
                                 lhsT=convd[:, (i * 9 + k) * 128:(i * 9 + k + 1) * 128],
                                 rhs=xa[:, base:base + PCOLS],
                                 start=(k == 0), stop=(k == 8))
            pv4 = cp[:, 0:PCOLS].rearrange("p (f r w) -> p f r w",
                                           r=10, w=PADW)[:, :, 1:9, 1:9]
            zc4 = zc[:, c * ZCOLS:(c + 1) * ZCOLS].rearrange(
                "p (f h w) -> p f h w", h=8, w=8)
            nc.scalar.activation(zc4, pv4, ACTF.Copy,
                                 accum_out=sump_c[:, i * 16 + c:i * 16 + c + 1])
            sq = scr_p.tile([128, ZCOLS], F16, tag="sqcv", name=f"sqc{i}{c}")
            nc.vector.scalar_tensor_tensor(
                sq[:], zc[:, c * ZCOLS:(c + 1) * ZCOLS], 1.0,
                zc[:, c * ZCOLS:(c + 1) * ZCOLS], ALU.mult, ALU.mult,
                accum_out=sqp_c[:, i * 16 + c:i * 16 + c + 1])
        z_conv.append(zc)
    emit_ar("dw", 0, 6)
    emit_params("dw", 0, 6)
    for i in range(6):
        dump_rows("z_conv", 128 * i, z_conv[i])

    # ============ PHASE 5b: x2-LIF + conv-LIF + gating ============
    gated = []
    for i in range(6):
        zx2 = pE.tile([128, R], F16, tag="zx2r", name=f"zx2r{i}", bufs=2)
        nc.sync.dma_start(zx2[:], zx2_sp[128 * i:128 * (i + 1), :])
        sx2 = pE.tile([128, R], F16, tag="gt", name=f"sx2_{i}", bufs=2)
        emit_lif(zx2, spike_writer(sx2), name="fc1", pt=6 + i)
        scv = pE.tile([128, R], F16, tag="scv", name=f"scv{i}", bufs=2)
        emit_lif(z_conv[i], spike_writer(scv), name="dw", pt=i)
        dump_rows("s_conv", 128 * i, scv)
        g = pZ.tile([128, R], F16, tag=f"zf1_{i}", name=f"gated{i}")
        nc.vector.tensor_tensor(g[:], scv[:], sx2[:], ALU.mult)
        gated.append(g)
        dump_rows("gated", 128 * i, g)
    ctxE.close()
    if stop_after == 'conv':
        ctxZ.close(); ctxL.close(); return

    # ============ PHASE 6: fc2 + final residual ============
    ctxG = ExitStack()
    pG = ctxG.enter_context(tc.tile_pool(name="pG", bufs=1))
    wt_fc2 = []
    for i in range(6):
        w = pG.tile([128, C], F16, tag=f"wfc2_{i}", name=f"wfc2_{i}")
        nc.sync.dma_start(w[:], w_in["fc2"][128 * i:128 * (i + 1), :])
        wt_fc2.append(w)
    zf2 = []

    def zalloc_g(co):
        z = pG.tile([128, R], F16, tag=f"zf2{co}", name=f"zf2{co}")
        zf2.append(z)
        return z

    emit_linear("fc2", wt_fc2, gated, 0, 3, 6, zalloc_g, lambda co, z: None)
    emit_ar("fc2", 0, 3)
    emit_params("fc2", 0, 3)
    for pt in range(3):
        dump_rows("z_fc2", 128 * pt, zf2[pt])
        so = pG.tile([128, R], F16, tag="so", name=f"so{pt}", bufs=2)
        emit_lif(zf2[pt], spike_writer(so), name="fc2", pt=pt)
        xm = pG.tile([128, R], F32, tag=f"xmr{pt % 2}", name=f"xmr{pt}")
        nc.sync.dma_start(xm[:], xmid_sp[128 * pt:128 * (pt + 1), :])
        nc.vector.tensor_tensor(xm[:], xm[:], so[:], ALU.add)
        nc.sync.dma_start(out_d[128 * pt:128 * (pt + 1), :], xm[:])
    if tok_d is not None:
        tk = pG.tile([128, 1], F32, tag="tok", name="tk")
        nc.vector.memset(tk[:], 1.0)
        nc.sync.dma_start(tok_d[:], tk[:])
    ctxG.close()
    ctxZ.close()
    ctxL.close()


# ---------------- host glue ----------------

def _prep_inputs(inputs):
    x = np.asarray(inputs['x'], np.float32)
    xr = x.reshape(T, B, N, C)
    ident = np.eye(128, dtype=np.float16)
    mask = np.zeros((128, 512), np.float16)
    for blk in range(4):
        for h in range(4):
            mask[h * 32:(h + 1) * 32,
                 blk * 128 + h * 32:blk * 128 + (h + 1) * 32] = 0.125
    common = {"ident": ident, "mask": mask}
    for name in ("q", "k", "v", "p", "fc1", "fc2"):
        wdt = np.float16 if name in ("p", "fc2") else np.float32
        common[f"w_{name}"] = np.ascontiguousarray(
            np.asarray(inputs[name + "_w"]).T).astype(wdt)
    for name in ("q", "k", "v", "p", "fc1", "fc2"):
        common[f"pv_{name}"] = np.ascontiguousarray(np.stack(
            [np.asarray(inputs[name + "_g"], np.float32),
             np.asarray(inputs[name + "_be"], np.float32)], 1))
    common["pv_dw"] = np.ascontiguousarray(np.stack(
        [np.asarray(inputs["dw_g"], np.float32),
         np.asarray(inputs["dw_be"], np.float32)], 1))
    kv = np.asarray(inputs["dw_k"], np.float32).reshape(CH, 9)
    convd = np.zeros((54 * 128, 128), np.float16)
    for i in range(6):
        for k in range(9):
            blk = np.diag(kv[i * 128:(i + 1) * 128, k]).astype(np.float16)
            convd[(i * 9 + k) * 128:(i * 9 + k + 1) * 128, :] = blk
    common["convd"] = convd

    maps = []
    for c in range(NCORES):
        shard = xr[:, c * BS:(c + 1) * BS]
        xt = np.ascontiguousarray(shard.reshape(R, C).T)
        m = dict(common)
        m["xT"] = xt
        maps.append(m)
    return maps


_CACHE = {}


def _get_runner(debug_taps=False, timing=False, stop_after=None):
    key = (debug_taps, timing, stop_after)
    if key not in _CACHE:
        from runner_embed import SpmdRunner
        nc = build_kernel(debug_taps, timing, stop_after)
        _CACHE[key] = SpmdRunner(nc, NCORES)
    return _CACHE[key]


def kernel(**inputs):
    r = _get_runner()
    maps = _prep_inputs(inputs)
    args = r.prep(maps)
    outs = r.run(args)
    res = r.results(outs)
    full = np.empty((T, B, N, C), np.float32)
    for c in range(NCORES):
        o = res[c]["out"]
        full[:, c * BS:(c + 1) * BS] = o.T.reshape(T, BS, N, C)
    return np.ascontiguousarray(full.reshape(T * B, N, C))


# ---- embedded SPMD runner module ----
import types
runner_embed = types.ModuleType("runner_embed")
sys.modules["runner_embed"] = runner_embed
exec(r'''
import sys
sys.path.insert(0, '/opt/trn_rl_repo')
import numpy as np
import jax
from jax.sharding import Mesh, PartitionSpec, NamedSharding
from jax.experimental.shard_map import shard_map
import concourse.bass as bass
import concourse.mybir as mybir
from concourse.bass2jax import _bass_exec_p, install_neuronx_cc_hook, partition_id_tensor


class SpmdRunner:
    def __init__(self, nc, n_cores):
        install_neuronx_cc_hook()
        self.nc = nc
        self.n_cores = n_cores
        partition_name = nc.partition_id_tensor.name if nc.partition_id_tensor else None
        in_names, out_names, out_avals, zero_outs = [], [], [], []
        for alloc in nc.m.functions[0].allocations:
            if not isinstance(alloc, mybir.MemoryLocationSet):
                continue
            name = alloc.memorylocations[0].name
            if alloc.kind == "ExternalInput":
                if name != partition_name:
                    in_names.append(name)
            elif alloc.kind == "ExternalOutput":
                shape = tuple(alloc.tensor_shape)
                dtype = mybir.dt.np(alloc.dtype)
                out_names.append(name)
                out_avals.append(jax.core.ShapedArray(shape, dtype))
                zero_outs.append(np.zeros(shape, dtype))
        self.in_names, self.out_names = in_names, out_names
        self.out_avals, self.zero_outs = out_avals, zero_outs
        n_params = len(in_names)
        n_outs = len(out_avals)
        all_in_names = list(in_names) + list(out_names)
        if partition_name is not None:
            all_in_names.append(partition_name)
        self.n_params = n_params

        def _body(*args):
            operands = list(args)
            if partition_name is not None:
                operands.append(partition_id_tensor())
            outs = _bass_exec_p.bind(
                *operands, out_avals=tuple(out_avals),
                in_names=tuple(all_in_names), out_names=tuple(out_names),
                lowering_input_output_aliases=(),
                sim_require_finite=True, sim_require_nnan=True, nc=nc)
            return tuple(outs)

        devices = jax.devices()[:n_cores]
        assert len(devices) == n_cores
        mesh = Mesh(np.asarray(devices), ("core",))
        self.mesh = mesh
        in_specs = (PartitionSpec("core"),) * (n_params + n_outs)
        out_specs = (PartitionSpec("core"),) * n_outs
        self.fn = jax.jit(
            shard_map(_body, mesh=mesh, in_specs=in_specs,
                      out_specs=out_specs, check_rep=False),
            keep_unused=True)

    def prep(self, in_maps):
        per_core = [[np.asarray(m[name]) for name in self.in_names]
                    for m in in_maps]
        concat_in = [np.concatenate([per_core[c][i] for c in range(self.n_cores)], axis=0)
                     for i in range(self.n_params)]
        concat_zeros = [np.zeros((self.n_cores * z.shape[0], *z.shape[1:]), z.dtype)
                        for z in self.zero_outs]
        sh = NamedSharding(self.mesh, PartitionSpec("core"))
        return [jax.device_put(a, sh) for a in concat_in + concat_zeros]

    def run(self, args):
        outs = self.fn(*args)
        jax.block_until_ready(outs)
        return outs

    def results(self, outs):
        res = []
        for c in range(self.n_cores):
            res.append({name: np.asarray(outs[i]).reshape(self.n_cores, *self.out_avals[i].shape)[c]
                        for i, name in enumerate(self.out_names)})
        return res

    def time_it(self, args, iters=20, warmup=3):
        import time
        for _ in range(warmup):
            self.run(args)
        ts = []
        for _ in range(iters):
            t0 = time.perf_counter()
            self.run(args)
            ts.append(time.perf_counter() - t0)
        ts = np.array(ts)
        return dict(min=ts.min(), median=float(np.median(ts)), mean=ts.mean())
''', runner_embed.__dict__)
